# revision 1
# baseline (speedup 1.0000x reference)
"""AdversarialMorphingLoss — Trainium2 Bass kernel (8-core data parallel).

Full inputs arrive on the host; we shard the batch dim (B=4096) into 8
contiguous blocks of 512 rows, run one SPMD Bass program on all 8
NeuronCores, and each core returns the partial (un-normalized) sum of the
per-sample loss contribution over its 512 rows.  The host sums the 8
partials and divides by B.

Per-sample math (matching reference.py):
  scores_b = 100/S * sum_s inc_s * CONFIG_MULT[pid % 4]
  inc_s    = 0.6*(sz_s > 1400) + 0.4*(dly_s < 0.05)
           + 0.2*(|sz_s - sz_{s-1}| < 0.5) + 0.1*(dir_s != dir_{s-1})
  with sz[:, -1] -> min(sz[:, -1] + pad*1500, 1500), dly[:, -1] += delay_ms,
  and the s=0 "prev" being -1.0 (so the dir term contributes 0.1 at s=0 and
  the size-equality term contributes 0).

  c_b = (2/30)*relu(scores-15) + 0.5*(|dly_ms - TD[pid]| + |pad - TP[pid]|)
      + 0.3*(relu(dly_ms-20)/20 + relu(pad-0.3)) + 0.2*(conf - (scores<30))^2
  loss = mean_b c_b

On-device strategy (memory-bound: streams 96 MB of traces):
  * count (sz > 1400) over all S int32 cols with one ScalarE
    activation(Sign, bias=-1400.5, accum_out=...) per [128, 2048] tile
    (integers never hit the .5 threshold -> exact), then patch the last
    (float-modified) column with exact [128,4] is_gt ops.
  * count (dly < 0.05) the same way via Sign(0.05 - dly).
  * count consecutive-size equality / direction flips with one fused
    VectorE tensor_tensor_reduce(is_equal / not_equal, accum_out=...) per
    tile, again patching the last column separately.
  * everything per-sample afterwards runs on tiny [128, 4] tiles.
"""

import numpy as np
from contextlib import ExitStack

import concourse.bass as bass
import concourse.bacc as bacc
import concourse.mybir as mybir
from concourse import tile
from concourse.bass_utils import run_bass_kernel_spmd

B, S = 4096, 2048
N_CORES = 8
BC = B // N_CORES          # 512 rows per core
P = 128                    # SBUF partitions
NT = BC // P               # 4 tiles of 128 rows per core

F32 = mybir.dt.float32
I32 = mybir.dt.int32
ALU = mybir.AluOpType
ACTF = mybir.ActivationFunctionType

_NC_CACHE = None
LAST_RESULTS = None        # BassKernelResults of the last kernel() call


def _patch_drain(tc, out_dma_holder):
    """Slim TileContext's exit drain (controlled by KERNEL_DRAIN_MODE):
    'full'     stock ending (drain + EVSEM barrier + sem clear + barrier)
    'nobar2'   stock minus the trailing all-engine barrier
    'plainsem' plain-semaphore ending: the output DMA already implies all
               prior work (every instruction feeds it transitively), so the
               sync engine waits for its completion semaphore, every engine
               bumps a barrier semaphore and halts, and gpsimd (last) waits
               for the barrier then clears/resets all semaphores for NEFF
               re-execution.  Avoids the slow event-semaphore butterflies.
    """
    import os
    import re
    import types
    from concourse.vector_clock import ScopedClock

    mode = os.environ.get("KERNEL_DRAIN_MODE", "nobar2")
    if mode == "full":
        return

    def _slim(self, tick_clock, wait_clock):
        nc = self.nc
        if mode == "plainsem":
            # Replicate the final drain with a chain of single-semaphore
            # plain waits on the sync engine (sum each semaphore's program-
            # wide updates and wait for that final value) instead of the
            # multi-wait NOP that Bacc lowers to a slow event-semaphore
            # butterfly.  Then a plain-semaphore all-engine barrier, then
            # gpsimd (which passes the barrier last, after sync has observed
            # every DMA completion) clears semaphores for NEFF re-execution.
            totals = {}
            upd_re = re.compile(r"update:S\[([A-Za-z0-9_]+)\](?:\+\+|\+=)(\d+)")
            for bb in nc.main_func.blocks:
                for ins in bb.instructions:
                    for mm in upd_re.finditer(str(ins)):
                        totals[mm.group(1)] = totals.get(mm.group(1), 0) + int(mm.group(2))
            by_name = {h.name: h for h in self.sems.allocated().values()}
            waits = [(h, totals[name]) for name, h in sorted(by_name.items())
                     if totals.get(name, 0) > 0]
            for eng in nc.engines.values():
                for h, total in waits:
                    eng.wait_ge(h, total)
            popped = nc._tile_sem_poison_stack.pop()
            assert popped is self._sem_poison
            nc.clear_and_free_semaphores(
                list(self.sems.allocated().values()))
            return
        drain_inst = nc.sync.drain()
        wait_clock.add_sem_waits(
            drain_inst.ins, ScopedClock({None: tick_clock.global_clock}))
        nc.all_engine_barrier()
        popped = nc._tile_sem_poison_stack.pop()
        assert popped is self._sem_poison
        nc.clear_and_free_semaphores(list(self.sems.allocated().values()))

    tc._drain_and_barrier = types.MethodType(_slim, tc)


def _build_nc() -> bass.Bass:
    nc = bacc.Bacc()

    sz_h = nc.declare_dram_parameter("raw_sizes", [BC, S], I32, isOutput=False)
    dl_h = nc.declare_dram_parameter("raw_delays", [BC, S], F32, isOutput=False)
    dr_h = nc.declare_dram_parameter("raw_directions", [BC, S], I32, isOutput=False)
    dms_h = nc.declare_dram_parameter("delay_ms", [BC], F32, isOutput=False)
    pad_h = nc.declare_dram_parameter("padding_norm", [BC], F32, isOutput=False)
    cnf_h = nc.declare_dram_parameter("confidence", [BC], F32, isOutput=False)
    pid_h = nc.declare_dram_parameter("profile_ids", [BC], I32, isOutput=False)
    out_h = nc.declare_dram_parameter("partial", [P, 1], F32, isOutput=True)

    out_dma_holder = []
    with tile.TileContext(nc) as tc, ExitStack() as ctx:
        _patch_drain(tc, out_dma_holder)
        inp = ctx.enter_context(tc.tile_pool(name="inp", bufs=4))
        scr = ctx.enter_context(tc.tile_pool(name="scr", bufs=2))
        sm = ctx.enter_context(tc.tile_pool(name="sm", bufs=1))

        def smt(tag, dtype=F32):
            return sm.tile([P, NT], dtype, tag=tag, name=tag)

        _consts = {}

        def constv(val):
            """[128,1] f32 SBUF tile holding `val` (for activation bias APs)."""
            if val not in _consts:
                cname = f"cst{len(_consts)}"
                ct = sm.tile([P, 1], F32, tag=cname, name=cname)
                nc.vector.memset(ct[:, :], val)
                _consts[val] = ct[:, :]
            return _consts[val]

        # Row mapping: core row r -> (partition p, tile t) with r = p*NT + t.
        # This makes the per-row [128, NT] vector loads a dense 2D DMA
        # (partition stride 16B) instead of a 512-descriptor gather, while
        # the big tile loads just become row-strided (stride NT rows), which
        # costs the same descriptors as contiguous rows.
        dvec = smt("dvec")
        pvec = smt("pvec")
        cvec = smt("cvec")
        pidt = smt("pidt", I32)

        # Big-op accumulators (per tile column), split into column halves so
        # the last tile's compute tail after its DMA is only a half-op deep
        R1a, R1b = smt("R1a"), smt("R1b")   # sum sign(sz - 1400.5)
        R2a, R2b = smt("R2a"), smt("R2b")   # sum sign(0.05 - dly)
        R3a, R3b = smt("R3a"), smt("R3b")   # count sz[s] == sz[s-1], s=1..S-1
        R4a, R4b = smt("R4a"), smt("R4b")   # count dir[s] != dir[s-1], s=1..S-1
        szlast = smt("szlast")   # f32 copy of int sz[:, S-1]
        szprev = smt("szprev")   # f32 copy of int sz[:, S-2]
        dllast = smt("dllast")   # f32 copy of dly[:, S-1]

        sz_t = sz_h[:, :].rearrange("(p t) s -> t p s", t=NT)
        dl_t = dl_h[:, :].rearrange("(p t) s -> t p s", t=NT)
        dr_t = dr_h[:, :].rearrange("(p t) s -> t p s", t=NT)
        H = S // 2
        for t in range(NT):
            szt = inp.tile([P, S], I32, tag="szt")
            dlt = inp.tile([P, S], F32, tag="dlt")
            drt = inp.tile([P, S], I32, tag="drt")
            for h, cs in ((0, slice(0, H)), (1, slice(H, S))):
                nc.sync.dma_start(szt[:, cs], sz_t[t][:, cs])
                nc.sync.dma_start(dlt[:, cs], dl_t[t][:, cs])
                nc.sync.dma_start(drt[:, cs], dr_t[t][:, cs])

            col = slice(t, t + 1)
            o1 = scr.tile([P, S], F32, tag="o1")
            nc.scalar.activation(o1[:, 0:H], szt[:, 0:H], ACTF.Sign,
                                 bias=constv(-1400.5), scale=1.0, accum_out=R1a[:, col])
            nc.scalar.activation(o1[:, H:S], szt[:, H:S], ACTF.Sign,
                                 bias=constv(-1400.5), scale=1.0, accum_out=R1b[:, col])
            o2 = scr.tile([P, S], F32, tag="o2")
            nc.scalar.activation(o2[:, 0:H], dlt[:, 0:H], ACTF.Sign,
                                 bias=constv(0.05), scale=-1.0, accum_out=R2a[:, col])
            nc.scalar.activation(o2[:, H:S], dlt[:, H:S], ACTF.Sign,
                                 bias=constv(0.05), scale=-1.0, accum_out=R2b[:, col])
            # fused compare + row-sum on DVE: out = (in0 bypass 0) cmp in1,
            # accum_out = sum(out).  (tensor_tensor_reduce crashes the HW
            # runtime in this toolchain; scalar_tensor_tensor w/ accum works.)
            o3 = scr.tile([P, S - 1], F32, tag="o3")
            nc.vector.scalar_tensor_tensor(
                o3[:, 0:H - 1], szt[:, 1:H], 0.0, szt[:, 0:H - 1],
                ALU.bypass, ALU.is_equal, accum_out=R3a[:, col])
            nc.vector.scalar_tensor_tensor(
                o3[:, H - 1:S - 1], szt[:, H:S], 0.0, szt[:, H - 1:S - 1],
                ALU.bypass, ALU.is_equal, accum_out=R3b[:, col])
            o4 = scr.tile([P, S - 1], F32, tag="o4")
            nc.vector.scalar_tensor_tensor(
                o4[:, 0:H - 1], drt[:, 1:H], 0.0, drt[:, 0:H - 1],
                ALU.bypass, ALU.not_equal, accum_out=R4a[:, col])
            nc.vector.scalar_tensor_tensor(
                o4[:, H - 1:S - 1], drt[:, H:S], 0.0, drt[:, H - 1:S - 1],
                ALU.bypass, ALU.not_equal, accum_out=R4b[:, col])

            nc.vector.tensor_copy(szlast[:, col], szt[:, S - 1:S])
            nc.vector.tensor_copy(szprev[:, col], szt[:, S - 2:S - 1])
            nc.vector.tensor_copy(dllast[:, col], dlt[:, S - 1:S])

        # per-row vectors loaded after the big streams are queued (tiny DMAs)
        nc.gpsimd.dma_start(dvec[:, :], dms_h[:].rearrange("(p t) -> p t", t=NT))
        nc.gpsimd.dma_start(pvec[:, :], pad_h[:].rearrange("(p t) -> p t", t=NT))
        nc.gpsimd.dma_start(cvec[:, :], cnf_h[:].rearrange("(p t) -> p t", t=NT))
        nc.gpsimd.dma_start(pidt[:, :], pid_h[:].rearrange("(p t) -> p t", t=NT))

        # ---- per-sample combine, all on [128, 4] tiles (VectorE only,
        # to keep per-instruction sync-wait counts low on ScalarE) ----
        v = nc.vector

        # merge column-half accumulators
        R1, R2, R3, R4 = smt("R1"), smt("R2"), smt("R3"), smt("R4")
        v.tensor_add(R1[:, :], R1a[:, :], R1b[:, :])
        v.tensor_add(R2[:, :], R2a[:, :], R2b[:, :])
        v.tensor_add(R3[:, :], R3a[:, :], R3b[:, :])
        v.tensor_add(R4[:, :], R4a[:, :], R4b[:, :])

        # profile-id one-hots (pid in 0..4)
        pidf = smt("pidf")
        v.tensor_copy(pidf[:, :], pidt[:, :])
        e1, e2, e3, e4 = smt("e1"), smt("e2"), smt("e3"), smt("e4")
        v.tensor_scalar(e1[:, :], pidf[:, :], 1.0, None, ALU.is_equal)
        v.tensor_scalar(e2[:, :], pidf[:, :], 2.0, None, ALU.is_equal)
        v.tensor_scalar(e3[:, :], pidf[:, :], 3.0, None, ALU.is_equal)
        v.tensor_scalar(e4[:, :], pidf[:, :], 4.0, None, ALU.is_equal)

        # CONFIG_MULT[pid % 4] = 1.0 + 0.3*e1 + 0.6*e2 + 1.0*e3  (pid=4 -> 1.0)
        mlt = smt("mlt")
        v.tensor_scalar(mlt[:, :], e1[:, :], 0.3, 1.0, ALU.mult, ALU.add)
        v.scalar_tensor_tensor(mlt[:, :], e2[:, :], 0.6, mlt[:, :], ALU.mult, ALU.add)
        v.tensor_add(mlt[:, :], mlt[:, :], e3[:, :])

        # TARGET_DELAY[pid] = 2 - 1*e1 - 1.5*e2 + 3*e3 + 1*e4
        td = smt("td")
        v.tensor_scalar(td[:, :], e1[:, :], -1.0, 2.0, ALU.mult, ALU.add)
        v.scalar_tensor_tensor(td[:, :], e2[:, :], -1.5, td[:, :], ALU.mult, ALU.add)
        v.scalar_tensor_tensor(td[:, :], e3[:, :], 3.0, td[:, :], ALU.mult, ALU.add)
        v.tensor_add(td[:, :], td[:, :], e4[:, :])

        # TARGET_PAD[pid] = 0.08 + 0.04*e1 - 0.03*e2 + 0.07*e3 + 0.02*e4
        tp = smt("tp")
        v.tensor_scalar(tp[:, :], e1[:, :], 0.04, 0.08, ALU.mult, ALU.add)
        v.scalar_tensor_tensor(tp[:, :], e2[:, :], -0.03, tp[:, :], ALU.mult, ALU.add)
        v.scalar_tensor_tensor(tp[:, :], e3[:, :], 0.07, tp[:, :], ALU.mult, ALU.add)
        v.scalar_tensor_tensor(tp[:, :], e4[:, :], 0.02, tp[:, :], ALU.mult, ALU.add)

        # last-column morphing fixups
        padx = smt("padx")
        v.tensor_scalar(padx[:, :], pvec[:, :], 1500.0, None, ALU.mult)
        szmod = smt("szmod")
        v.tensor_add(szmod[:, :], szlast[:, :], padx[:, :])
        v.tensor_scalar(szmod[:, :], szmod[:, :], 1500.0, None, ALU.min)
        dlmod = smt("dlmod")
        v.tensor_add(dlmod[:, :], dllast[:, :], dvec[:, :])

        g1m, g1r = smt("g1m"), smt("g1r")
        v.tensor_scalar(g1m[:, :], szmod[:, :], 1400.0, None, ALU.is_gt)
        v.tensor_scalar(g1r[:, :], szlast[:, :], 1400.0, None, ALU.is_gt)
        l2m, l2r = smt("l2m"), smt("l2r")
        v.tensor_scalar(l2m[:, :], dlmod[:, :], 0.05, None, ALU.is_lt)
        v.tensor_scalar(l2r[:, :], dllast[:, :], 0.05, None, ALU.is_lt)
        e3r = smt("e3r")
        v.tensor_tensor(e3r[:, :], szlast[:, :], szprev[:, :], ALU.is_equal)
        d3 = smt("d3")
        v.tensor_sub(d3[:, :], szmod[:, :], szprev[:, :])
        a3 = smt("a3")
        nc.scalar.activation(a3[:, :], d3[:, :], ACTF.Abs)
        e3m = smt("e3m")
        v.tensor_scalar(e3m[:, :], a3[:, :], 0.5, None, ALU.is_lt)

        # exact per-row counts
        cnt1 = smt("cnt1")
        v.tensor_scalar(cnt1[:, :], R1[:, :], 0.5, float(S) / 2, ALU.mult, ALU.add)
        v.tensor_sub(cnt1[:, :], cnt1[:, :], g1r[:, :])
        v.tensor_add(cnt1[:, :], cnt1[:, :], g1m[:, :])
        cnt2 = smt("cnt2")
        v.tensor_scalar(cnt2[:, :], R2[:, :], 0.5, float(S) / 2, ALU.mult, ALU.add)
        v.tensor_sub(cnt2[:, :], cnt2[:, :], l2r[:, :])
        v.tensor_add(cnt2[:, :], cnt2[:, :], l2m[:, :])
        cnt3 = smt("cnt3")
        v.tensor_sub(cnt3[:, :], R3[:, :], e3r[:, :])
        v.tensor_add(cnt3[:, :], cnt3[:, :], e3m[:, :])

        # scores = (0.6*c1 + 0.4*c2 + 0.2*c3 + 0.1*c4 + 0.1) * (100/S) * mult
        acc = smt("acc")
        v.tensor_scalar(acc[:, :], cnt1[:, :], 0.6, None, ALU.mult)
        v.scalar_tensor_tensor(acc[:, :], cnt2[:, :], 0.4, acc[:, :], ALU.mult, ALU.add)
        v.scalar_tensor_tensor(acc[:, :], cnt3[:, :], 0.2, acc[:, :], ALU.mult, ALU.add)
        v.scalar_tensor_tensor(acc[:, :], R4[:, :], 0.1, acc[:, :], ALU.mult, ALU.add)
        base = smt("base")
        v.tensor_scalar(base[:, :], acc[:, :], 100.0 / S, 0.1 * 100.0 / S,
                        ALU.mult, ALU.add)
        scores = smt("scores")
        v.tensor_mul(scores[:, :], base[:, :], mlt[:, :])

        ev = smt("ev")
        v.tensor_scalar(ev[:, :], scores[:, :], 30.0, None, ALU.is_lt)
        dpi = smt("dpi")
        v.tensor_scalar(dpi[:, :], scores[:, :], 15.0, -15.0, ALU.max, ALU.add)

        sd = smt("sd")
        v.tensor_sub(sd[:, :], dvec[:, :], td[:, :])
        sda = smt("sda")
        nc.scalar.activation(sda[:, :], sd[:, :], ACTF.Abs)
        sp = smt("sp")
        v.tensor_sub(sp[:, :], pvec[:, :], tp[:, :])
        spa = smt("spa")
        nc.scalar.activation(spa[:, :], sp[:, :], ACTF.Abs)
        sim = smt("sim")
        v.tensor_add(sim[:, :], sda[:, :], spa[:, :])

        ed = smt("ed")
        v.tensor_scalar(ed[:, :], dvec[:, :], 20.0, -20.0, ALU.max, ALU.add)
        ep = smt("ep")
        v.tensor_scalar(ep[:, :], pvec[:, :], 0.3, -0.3, ALU.max, ALU.add)
        eff = smt("eff")
        v.scalar_tensor_tensor(eff[:, :], ed[:, :], 1.0 / 20.0, ep[:, :],
                               ALU.mult, ALU.add)

        cd = smt("cd")
        v.tensor_sub(cd[:, :], cvec[:, :], ev[:, :])
        cq = smt("cq")
        v.tensor_mul(cq[:, :], cd[:, :], cd[:, :])

        ctot = smt("ctot")
        v.tensor_scalar(ctot[:, :], dpi[:, :], 2.0 / 30.0, None, ALU.mult)
        v.scalar_tensor_tensor(ctot[:, :], sim[:, :], 0.5, ctot[:, :], ALU.mult, ALU.add)
        v.scalar_tensor_tensor(ctot[:, :], eff[:, :], 0.3, ctot[:, :], ALU.mult, ALU.add)
        v.scalar_tensor_tensor(ctot[:, :], cq[:, :], 0.2, ctot[:, :], ALU.mult, ALU.add)

        red = sm.tile([P, 1], F32, tag="red", name="red")
        v.tensor_reduce(red[:, :], ctot[:, :], axis=mybir.AxisListType.X, op=ALU.add)
        out_dma_holder.append(nc.sync.dma_start(out_h[:, :], red[:, :]))

    nc.finalize()
    return nc


def _get_nc() -> bass.Bass:
    global _NC_CACHE
    if _NC_CACHE is None:
        _NC_CACHE = _build_nc()
    return _NC_CACHE


def kernel(raw_sizes, raw_delays, raw_directions, delay_ms, padding_norm,
           confidence, profile_ids, trace=False, tmpdir=None):
    global LAST_RESULTS
    raw_sizes = np.asarray(raw_sizes, dtype=np.int32)
    raw_delays = np.asarray(raw_delays, dtype=np.float32)
    raw_directions = np.asarray(raw_directions, dtype=np.int32)
    delay_ms = np.asarray(delay_ms, dtype=np.float32)
    padding_norm = np.asarray(padding_norm, dtype=np.float32)
    confidence = np.asarray(confidence, dtype=np.float32)
    profile_ids = np.asarray(profile_ids).astype(np.int32)

    nc = _get_nc()
    in_maps = []
    for i in range(N_CORES):
        r = slice(i * BC, (i + 1) * BC)
        in_maps.append({
            "raw_sizes": raw_sizes[r],
            "raw_delays": raw_delays[r],
            "raw_directions": raw_directions[r],
            "delay_ms": delay_ms[r],
            "padding_norm": padding_norm[r],
            "confidence": confidence[r],
            "profile_ids": profile_ids[r],
        })

    LAST_RESULTS = run_bass_kernel_spmd(nc, in_maps, list(range(N_CORES)),
                                        trace=trace, tmpdir=tmpdir)
    partials = [LAST_RESULTS.results[i]["partial"] for i in range(N_CORES)]
    total = float(np.sum(np.stack(partials), dtype=np.float64))
    return np.float32(total / B)



# revision 7
# speedup vs baseline: 1.6503x; 1.6503x over previous
"""AdversarialMorphingLoss — Trainium2 Bass kernel (8-core data parallel).

Full inputs arrive on the host; we shard the batch dim (B=4096) into 8
contiguous blocks of 512 rows, run one SPMD Bass program on all 8
NeuronCores, and each core returns the partial (un-normalized) sum of the
per-sample loss contribution over its 512 rows.  The host sums the 8
partials and divides by B.

Host-side prep (O(B) math + dtype casts):
  * the three [B, S] trace tensors are cast to fp16 (sizes/directions are
    exact; delays lose ~1e-4 relative — loss delta ~2e-7, tol 2e-2).
    Halves HBM traffic per core from 12.6 MB to 6.05 MB.
  * per-sample quantities depending only on [B] vectors are folded into a
    packed [B, 8] f32 tensor V:
      V0 = padding_norm*1500, V1 = delay_ms, V2 = CONFIG_MULT[pid%4]*100/S,
      V3 = 0.5*sim + 0.3*eff + 0.2*conf^2, V4 = 0.2*(1-2*conf)
    so ctot_b = (2/30)*relu(scores-15) + V3 + V4*(scores<30).

Device strategy (measured op rates on this toolchain):
  * DVE scalar_tensor_tensor w/ accum_out: 1x (2.29us per 2048-col tile)
    — the only fused compare+row-reduce DVE form that works on HW.
  * ScalarE ACTIVATE w/ accum_out: 1x @1.2GHz (2.0us) — Sign-based
    threshold counts.  16 fused count ops total, split 8/8:
      ACT: (sz>1400) and (dl<0.05) via Sign (sign-sum convention)
      DVE: sz[s]==sz[s-1] (is_equal), dir[s]!=dir[s-1] (not_equal)
  * all 16 input DMA triggers issue from the sync queue; tile 0's three
    tensors stream as 256KB halves so compute starts ~5us earlier.
  * every accumulator + last-col fixup lands in a [128, 64] f32 Rblock
    (16 slots x 4 tiles; slots 12-15 hold tile-0's second halves); the
    whole weighted merge is one tensor_tensor against a memset W tile +
    one strided tensor_reduce.
  * partition reduction via TensorE matmul into PSUM so the output DMA
    is one 4-byte descriptor (a [128,1] scatter costs ~4us completion).
"""

import numpy as np
from contextlib import ExitStack

import concourse.bass as bass
import concourse.bacc as bacc
import concourse.mybir as mybir
from concourse import tile
from concourse.bass_utils import run_bass_kernel_spmd

B, S = 4096, 2048
N_CORES = 8
BC = B // N_CORES          # 512 rows per core
P = 128                    # SBUF partitions
NT = BC // P               # 4 tiles of 128 rows per core
NR = 16                    # Rblock slots per tile
H = S // 2

F32 = mybir.dt.float32
F16 = mybir.dt.float16
ALU = mybir.AluOpType
ACTF = mybir.ActivationFunctionType

# per-profile targets (match reference.py)
TARGET_DELAY = np.array([2.0, 1.0, 0.5, 5.0, 3.0], dtype=np.float32)
TARGET_PAD = np.array([0.08, 0.12, 0.05, 0.15, 0.10], dtype=np.float32)
CONFIG_MULT = np.array([1.0, 1.3, 1.6, 2.0], dtype=np.float32)

_NC_CACHE = None
LAST_RESULTS = None        # BassKernelResults of the last kernel() call


def _patch_drain(tc, out_dma_holder):
    """Slim TileContext's exit drain (controlled by KERNEL_DRAIN_MODE):
    'full'     stock ending (drain + EVSEM barrier + sem clear + barrier)
    'nobar2'   stock minus the trailing all-engine barrier
    'plainsem' plain-semaphore ending (see baseline notes).
    """
    import os
    import re
    import types
    from concourse.vector_clock import ScopedClock

    mode = os.environ.get("KERNEL_DRAIN_MODE", "nobar2")
    if mode == "full":
        return

    def _slim(self, tick_clock, wait_clock):
        nc = self.nc
        if mode == "plainsem":
            totals = {}
            upd_re = re.compile(r"update:S\[([A-Za-z0-9_]+)\](?:\+\+|\+=)(\d+)")
            for bb in nc.main_func.blocks:
                for ins in bb.instructions:
                    for mm in upd_re.finditer(str(ins)):
                        totals[mm.group(1)] = totals.get(mm.group(1), 0) + int(mm.group(2))
            by_name = {h.name: h for h in self.sems.allocated().values()}
            waits = [(h, totals[name]) for name, h in sorted(by_name.items())
                     if totals.get(name, 0) > 0]
            for eng in nc.engines.values():
                for h, total in waits:
                    eng.wait_ge(h, total)
            popped = nc._tile_sem_poison_stack.pop()
            assert popped is self._sem_poison
            nc.clear_and_free_semaphores(
                list(self.sems.allocated().values()))
            return
        drain_inst = nc.sync.drain()
        wait_clock.add_sem_waits(
            drain_inst.ins, ScopedClock({None: tick_clock.global_clock}))
        nc.all_engine_barrier()
        popped = nc._tile_sem_poison_stack.pop()
        assert popped is self._sem_poison
        nc.clear_and_free_semaphores(list(self.sems.allocated().values()))

    tc._drain_and_barrier = types.MethodType(_slim, tc)


def _build_nc() -> bass.Bass:
    nc = bacc.Bacc()

    sz_h = nc.declare_dram_parameter("raw_sizes", [BC, S], F16, isOutput=False)
    dl_h = nc.declare_dram_parameter("raw_delays", [BC, S], F16, isOutput=False)
    dr_h = nc.declare_dram_parameter("raw_directions", [BC, S], F16, isOutput=False)
    v_h = nc.declare_dram_parameter("vpack", [BC, 8], F32, isOutput=False)
    out_h = nc.declare_dram_parameter("partial", [1, 1], F32, isOutput=True)

    out_dma_holder = []
    with tile.TileContext(nc) as tc, ExitStack() as ctx:
        _patch_drain(tc, out_dma_holder)
        sm = ctx.enter_context(tc.tile_pool(name="sm", bufs=1))
        scr = ctx.enter_context(tc.tile_pool(name="scr", bufs=2))
        pp = ctx.enter_context(tc.tile_pool(name="pp", bufs=1, space="PSUM"))

        # big input tensors: one SBUF tensor per input, tile t = cols [t*S,(t+1)*S)
        SZ = sm.tile([P, NT * S], F16, tag="SZ", name="SZ")
        DL = sm.tile([P, NT * S], F16, tag="DL", name="DL")
        DR = sm.tile([P, NT * S], F16, tag="DR", name="DR")
        V = sm.tile([P, NT * 8], F32, tag="V", name="V")
        Rb = sm.tile([P, NT * NR], F32, tag="Rb", name="Rb")
        W = sm.tile([P, NT * NR], F32, tag="W", name="W")

        _consts = {}

        def constv(val):
            if val not in _consts:
                cname = f"cst{len(_consts)}"
                ct = sm.tile([P, 1], F32, tag=cname, name=cname)
                nc.vector.memset(ct[:, :], val)
                _consts[val] = ct[:, :]
            return _consts[val]

        # DRAM views: tile t holds rows r = p*NT + t
        sz_t = sz_h[:, :].rearrange("(p t) s -> t p s", t=NT)
        dl_t = dl_h[:, :].rearrange("(p t) s -> t p s", t=NT)
        dr_t = dr_h[:, :].rearrange("(p t) s -> t p s", t=NT)
        v_d = v_h[:, :].rearrange("(p t) v -> p (t v)", t=NT)

        def szs(t):
            return slice(t * S, (t + 1) * S)

        # ---- DMA triggers: all on the sync HWDGE ring, arrival order =
        # compute order; tile 0 streams as halves for a fast pipeline fill.
        nc.sync.dma_start(V[:, :], v_d)
        nc.sync.dma_start(SZ[:, 0:H], sz_t[0][:, 0:H])
        nc.sync.dma_start(SZ[:, H:S], sz_t[0][:, H:S])
        nc.sync.dma_start(DL[:, 0:H], dl_t[0][:, 0:H])
        nc.sync.dma_start(DL[:, H:S], dl_t[0][:, H:S])
        nc.sync.dma_start(DR[:, 0:H], dr_t[0][:, 0:H])
        nc.sync.dma_start(DR[:, H:S], dr_t[0][:, H:S])
        nc.sync.dma_start(SZ[:, szs(1)], sz_t[1])
        nc.sync.dma_start(DL[:, szs(1)], dl_t[1])
        nc.sync.dma_start(DR[:, szs(1)], dr_t[1])
        nc.sync.dma_start(SZ[:, szs(2)], sz_t[2])
        nc.sync.dma_start(DL[:, szs(2)], dl_t[2])
        nc.sync.dma_start(DR[:, szs(2)], dr_t[2])
        nc.sync.dma_start(SZ[:, szs(3)], sz_t[3])
        nc.sync.dma_start(DL[:, szs(3)], dl_t[3])
        nc.sync.dma_start(DR[:, szs(3)], dr_t[3])

        # ---- W weight tile + Rblock const columns (gpsimd memsets) ----
        # Rblock slot r semantics (per tile-column t):
        #  0: A = sum sign(sz-1400.5)        w=0.3   (0.6 * 1/2)
        #  1: B = sum sign(0.05-dl)          w=0.2   (0.4 * 1/2)
        #  2: C = sum is_equal(sz_s,sz_s-1)  w=0.2
        #  3: D = sum not_equal(dr_s,dr_s-1) w=0.1
        #  4: g1r=szlast>1400  w=-0.6    5: g1m=szmod>1400  w=+0.6
        #  6: l2r=dllast<0.05  w=-0.4    7: l2m=dlmod<0.05  w=+0.4
        #  8: e3r=szlast==szprev w=-0.2  9: e3m=|szmod-szprev|<0.5 w=+0.2
        # 10: const 1.0, w = 0.6*1024 + 0.4*1024 + 0.1   11: unused
        # 12-15: tile-0 second halves of slots 0-3 (same weights)
        Wr = W[:, :].rearrange("p (t r) -> r p t", r=NR)
        Rr = Rb[:, :].rearrange("p (t r) -> r p t", r=NR)
        g = nc.gpsimd
        g.memset(Wr[0], 0.3)
        g.memset(Wr[1], 0.2)
        g.memset(Wr[2], 0.2)
        g.memset(Wr[3], 0.1)
        g.memset(Wr[4], -0.6)
        g.memset(Wr[5], 0.6)
        g.memset(Wr[6], -0.4)
        g.memset(Wr[7], 0.4)
        g.memset(Wr[8], -0.2)
        g.memset(Wr[9], 0.2)
        g.memset(Wr[10], 0.6 * 1024.0 + 0.4 * 1024.0 + 0.1)
        g.memset(Wr[11], 0.0)
        g.memset(Rr[10], 1.0)
        g.memset(Rr[11], 0.0)
        g.memset(Wr[12][:, 0:1], 0.3)
        g.memset(Wr[13][:, 0:1], 0.2)
        g.memset(Wr[14][:, 0:1], 0.2)
        g.memset(Wr[15][:, 0:1], 0.1)
        g.memset(Wr[12][:, 1:4], 0.0)
        g.memset(Wr[13][:, 1:4], 0.0)
        g.memset(Wr[14][:, 1:4], 0.0)
        g.memset(Wr[15][:, 1:4], 0.0)
        for r in range(12, 16):
            g.memset(Rr[r][:, 1:4], 0.0)

        v = nc.vector

        def rslot(t, r):
            c = t * NR + r
            return Rb[:, c:c + 1]

        # ---- fused count ops (accum_out -> Rblock) ----
        def act_sign_sz(cs, slot):
            o = scr.tile([P, cs.stop - cs.start], F16, tag="osg")
            nc.scalar.activation(o[:, :], SZ[:, cs], ACTF.Sign,
                                 bias=constv(-1400.5), scale=1.0,
                                 accum_out=slot)

        def act_sign_dl(cs, slot):
            o = scr.tile([P, cs.stop - cs.start], F16, tag="osg")
            nc.scalar.activation(o[:, :], DL[:, cs], ACTF.Sign,
                                 bias=constv(0.05), scale=-1.0,
                                 accum_out=slot)

        def dve_eq_sz(lo, hi, slot):
            o = scr.tile([P, hi - lo], F16, tag="oeq")
            v.scalar_tensor_tensor(
                o[:, :], SZ[:, lo + 1:hi + 1], 0.0, SZ[:, lo:hi],
                ALU.bypass, ALU.is_equal, accum_out=slot)

        def dve_ne_dr(lo, hi, slot):
            o = scr.tile([P, hi - lo], F16, tag="one")
            v.scalar_tensor_tensor(
                o[:, :], DR[:, lo + 1:hi + 1], 0.0, DR[:, lo:hi],
                ALU.bypass, ALU.not_equal, accum_out=slot)

        # scalar engine queue, in data-arrival order
        act_sign_sz(slice(0, H), rslot(0, 0))
        act_sign_sz(slice(H, S), rslot(0, 12))
        act_sign_dl(slice(0, H), rslot(0, 1))
        act_sign_dl(slice(H, S), rslot(0, 13))
        act_sign_sz(slice(S, 2 * S), rslot(1, 0))
        act_sign_dl(slice(S, 2 * S), rslot(1, 1))
        act_sign_sz(slice(2 * S, 3 * S), rslot(2, 0))
        act_sign_dl(slice(2 * S, 3 * S), rslot(2, 1))
        act_sign_sz(slice(3 * S, 4 * S), rslot(3, 0))
        act_sign_dl(slice(3 * S, 4 * S), rslot(3, 1))

        # vector engine queue, in data-arrival order
        dve_eq_sz(0, H - 1, rslot(0, 2))            # pairs s=1..H-1
        dve_eq_sz(H - 1, S - 1, rslot(0, 14))       # pairs s=H..S-1
        dve_ne_dr(0, H - 1, rslot(0, 3))
        dve_ne_dr(H - 1, S - 1, rslot(0, 15))
        dve_eq_sz(S, 2 * S - 1, rslot(1, 2))
        dve_ne_dr(S, 2 * S - 1, rslot(1, 3))
        dve_eq_sz(2 * S, 3 * S - 1, rslot(2, 2))
        dve_ne_dr(2 * S, 3 * S - 1, rslot(2, 3))
        dve_eq_sz(3 * S, 4 * S - 1, rslot(3, 2))
        dve_ne_dr(3 * S, 4 * S - 1, rslot(3, 3))

        # ---- per-sample fixups ([128, NT] strided views) ----
        SZr = SZ[:, :].rearrange("p (t s) -> s p t", s=S)
        DLr = DL[:, :].rearrange("p (t s) -> s p t", s=S)
        Vr = V[:, :].rearrange("p (t v) -> v p t", v=8)
        szlast, szprev, dllast = SZr[S - 1], SZr[S - 2], DLr[S - 1]
        padxv, dladdv, mltv, e0v, w1v = Vr[0], Vr[1], Vr[2], Vr[3], Vr[4]

        szmod = sm.tile([P, NT], F32, tag="szmod", name="szmod")
        v.tensor_tensor(szmod[:, :], szlast, padxv, ALU.add)
        v.tensor_scalar(szmod[:, :], szmod[:, :], 1500.0, None, ALU.min)
        dlmod = sm.tile([P, NT], F32, tag="dlmod", name="dlmod")
        v.tensor_tensor(dlmod[:, :], dllast, dladdv, ALU.add)

        v.tensor_scalar(Rr[4], szlast, 1400.0, None, ALU.is_gt)
        v.tensor_scalar(Rr[5], szmod[:, :], 1400.0, None, ALU.is_gt)
        v.tensor_scalar(Rr[6], dllast, 0.05, None, ALU.is_lt)
        v.tensor_scalar(Rr[7], dlmod[:, :], 0.05, None, ALU.is_lt)
        v.tensor_tensor(Rr[8], szlast, szprev, ALU.is_equal)
        d3 = sm.tile([P, NT], F32, tag="d3", name="d3")
        v.tensor_tensor(d3[:, :], szmod[:, :], szprev, ALU.subtract)
        a3 = sm.tile([P, NT], F32, tag="a3", name="a3")
        nc.scalar.activation(a3[:, :], d3[:, :], ACTF.Abs)
        v.tensor_scalar(Rr[9], a3[:, :], 0.5, None, ALU.is_lt)

        # ---- merge: scores per sample, then loss terms ----
        M = sm.tile([P, NT * NR], F32, tag="M", name="M")
        v.tensor_tensor(M[:, :], Rb[:, :], W[:, :], ALU.mult)
        s0 = sm.tile([P, NT], F32, tag="s0", name="s0")
        v.tensor_reduce(s0[:, :], M[:, :].rearrange("p (t r) -> p t r", r=NR),
                        axis=mybir.AxisListType.X, op=ALU.add)
        scores = sm.tile([P, NT], F32, tag="scores", name="scores")
        v.tensor_tensor(scores[:, :], s0[:, :], mltv, ALU.mult)

        ev = sm.tile([P, NT], F32, tag="ev", name="ev")
        v.tensor_scalar(ev[:, :], scores[:, :], 30.0, None, ALU.is_lt)
        dpi = sm.tile([P, NT], F32, tag="dpi", name="dpi")
        v.tensor_scalar(dpi[:, :], scores[:, :], 15.0, -15.0, ALU.max, ALU.add)

        u1 = sm.tile([P, NT], F32, tag="u1", name="u1")
        v.tensor_tensor(u1[:, :], ev[:, :], w1v, ALU.mult)
        v.tensor_tensor(u1[:, :], u1[:, :], e0v, ALU.add)
        ctot = sm.tile([P, NT], F32, tag="ctot", name="ctot")
        v.scalar_tensor_tensor(ctot[:, :], dpi[:, :], 2.0 / 30.0, u1[:, :],
                               ALU.mult, ALU.add)

        red = sm.tile([P, 1], F32, tag="red", name="red")
        v.tensor_reduce(red[:, :], ctot[:, :], axis=mybir.AxisListType.X,
                        op=ALU.add)

        # partition reduction on the (idle) tensor engine -> [1,1] PSUM
        ps = pp.tile([1, 1], F32, tag="ps", name="ps")
        nc.tensor.matmul(ps[:, :], red[:, :], constv(1.0), start=True, stop=True)
        outsb = sm.tile([1, 1], F32, tag="outsb", name="outsb")
        v.tensor_copy(outsb[:, :], ps[:, :])
        out_dma_holder.append(nc.sync.dma_start(out_h[:, :], outsb[:, :]))

    nc.finalize()
    return nc


def _get_nc() -> bass.Bass:
    global _NC_CACHE
    if _NC_CACHE is None:
        _NC_CACHE = _build_nc()
    return _NC_CACHE


def kernel(raw_sizes, raw_delays, raw_directions, delay_ms, padding_norm,
           confidence, profile_ids, trace=False, tmpdir=None):
    global LAST_RESULTS
    sz16 = np.asarray(raw_sizes).astype(np.float16)
    dl16 = np.asarray(raw_delays, dtype=np.float32).astype(np.float16)
    dr16 = np.asarray(raw_directions).astype(np.float16)
    dms = np.asarray(delay_ms, dtype=np.float32)
    pad = np.asarray(padding_norm, dtype=np.float32)
    conf = np.asarray(confidence, dtype=np.float32)
    pid = np.asarray(profile_ids).astype(np.int64)

    td = TARGET_DELAY[pid]
    tp = TARGET_PAD[pid]
    mult = CONFIG_MULT[pid % 4]
    sim = np.abs(dms - td) + np.abs(pad - tp)
    eff = np.maximum(dms - 20.0, 0.0) / 20.0 + np.maximum(pad - 0.3, 0.0)
    vpack = np.zeros((B, 8), dtype=np.float32)
    vpack[:, 0] = pad * 1500.0
    vpack[:, 1] = dms
    vpack[:, 2] = mult * (100.0 / S)
    vpack[:, 3] = 0.5 * sim + 0.3 * eff + 0.2 * conf * conf
    vpack[:, 4] = 0.2 * (1.0 - 2.0 * conf)

    nc = _get_nc()
    in_maps = []
    for i in range(N_CORES):
        r = slice(i * BC, (i + 1) * BC)
        in_maps.append({
            "raw_sizes": sz16[r],
            "raw_delays": dl16[r],
            "raw_directions": dr16[r],
            "vpack": vpack[r],
        })

    LAST_RESULTS = run_bass_kernel_spmd(nc, in_maps, list(range(N_CORES)),
                                        trace=trace, tmpdir=tmpdir)
    partials = [LAST_RESULTS.results[i]["partial"] for i in range(N_CORES)]
    total = float(np.sum(np.stack(partials), dtype=np.float64))
    return np.float32(total / B)


# revision 10
# speedup vs baseline: 1.7149x; 1.0392x over previous
"""AdversarialMorphingLoss — Trainium2 Bass kernel (8-core data parallel).

Full inputs arrive on the host; we shard the batch dim (B=4096) into 8
contiguous blocks of 512 rows, run one SPMD Bass program on all 8
NeuronCores, and each core returns the partial (un-normalized) sum of the
per-sample loss contribution over its 512 rows.  The host sums the 8
partials and divides by B.

Host-side prep (O(B) math + dtype casts):
  * the three [B, S] trace tensors are cast to fp16 (sizes/directions are
    exact; delays lose ~1e-4 relative — loss delta ~2e-7, tol 2e-2).
    Halves HBM traffic per core from 12.6 MB to 6.05 MB.
  * per-sample quantities depending only on [B] vectors are folded into a
    packed [B, 8] f32 tensor V:
      V0 = padding_norm*1500, V1 = delay_ms, V2 = CONFIG_MULT[pid%4]*100/S,
      V3 = 0.5*sim + 0.3*eff + 0.2*conf^2, V4 = 0.2*(1-2*conf)
    so ctot_b = (2/30)*relu(scores-15) + V3 + V4*(scores<30).

Device strategy (measured op rates on this toolchain):
  * DVE scalar_tensor_tensor w/ accum_out: 1x (2.29us per 2048-col tile)
    — the only fused compare+row-reduce DVE form that works on HW.
  * ScalarE ACTIVATE w/ accum_out: 1x @1.2GHz (2.0us) — Sign-based
    threshold counts.  16 fused count ops total, split 8/8:
      ACT: (sz>1400) and (dl<0.05) via Sign (sign-sum convention)
      DVE: sz[s]==sz[s-1] (is_equal), dir[s]!=dir[s-1] (not_equal)
  * all 16 input DMA triggers issue from the sync queue; tile 0's three
    tensors stream as 256KB halves so compute starts ~5us earlier.
  * every accumulator + last-col fixup lands in a [128, 64] f32 Rblock
    (16 slots x 4 tiles; slots 12-15 hold tile-0's second halves); the
    whole weighted merge is one tensor_tensor against a memset W tile +
    one strided tensor_reduce.
  * partition reduction via TensorE matmul into PSUM so the output DMA
    is one 4-byte descriptor (a [128,1] scatter costs ~4us completion).
"""

import numpy as np
from contextlib import ExitStack

import concourse.bass as bass
import concourse.bacc as bacc
import concourse.mybir as mybir
from concourse import tile
from concourse.bass_utils import run_bass_kernel_spmd

B, S = 4096, 2048
N_CORES = 8
BC = B // N_CORES          # 512 rows per core
P = 128                    # SBUF partitions
NT = BC // P               # 4 tiles of 128 rows per core
NR = 16                    # Rblock slots per tile
H = S // 2

F32 = mybir.dt.float32
F16 = mybir.dt.float16
ALU = mybir.AluOpType
ACTF = mybir.ActivationFunctionType

# per-profile targets (match reference.py)
TARGET_DELAY = np.array([2.0, 1.0, 0.5, 5.0, 3.0], dtype=np.float32)
TARGET_PAD = np.array([0.08, 0.12, 0.05, 0.15, 0.10], dtype=np.float32)
CONFIG_MULT = np.array([1.0, 1.3, 1.6, 2.0], dtype=np.float32)

_NC_CACHE = None
LAST_RESULTS = None        # BassKernelResults of the last kernel() call


def _patch_drain(tc, out_dma_holder):
    """Slim TileContext's exit drain (controlled by KERNEL_DRAIN_MODE):
    'full'     stock ending (drain + EVSEM barrier + sem clear + barrier)
    'nobar2'   stock minus the trailing all-engine barrier
    'plainsem' plain-semaphore ending (see baseline notes).
    """
    import os
    import re
    import types
    from concourse.vector_clock import ScopedClock

    mode = os.environ.get("KERNEL_DRAIN_MODE", "nobar2")
    if mode == "full":
        return

    def _slim(self, tick_clock, wait_clock):
        nc = self.nc
        if mode == "plainsem":
            totals = {}
            upd_re = re.compile(r"update:S\[([A-Za-z0-9_]+)\](?:\+\+|\+=)(\d+)")
            for bb in nc.main_func.blocks:
                for ins in bb.instructions:
                    for mm in upd_re.finditer(str(ins)):
                        totals[mm.group(1)] = totals.get(mm.group(1), 0) + int(mm.group(2))
            by_name = {h.name: h for h in self.sems.allocated().values()}
            waits = [(h, totals[name]) for name, h in sorted(by_name.items())
                     if totals.get(name, 0) > 0]
            for eng in nc.engines.values():
                for h, total in waits:
                    eng.wait_ge(h, total)
            popped = nc._tile_sem_poison_stack.pop()
            assert popped is self._sem_poison
            nc.clear_and_free_semaphores(
                list(self.sems.allocated().values()))
            return
        drain_inst = nc.sync.drain()
        wait_clock.add_sem_waits(
            drain_inst.ins, ScopedClock({None: tick_clock.global_clock}))
        nc.all_engine_barrier()
        popped = nc._tile_sem_poison_stack.pop()
        assert popped is self._sem_poison
        nc.clear_and_free_semaphores(list(self.sems.allocated().values()))

    tc._drain_and_barrier = types.MethodType(_slim, tc)


def _build_nc() -> bass.Bass:
    nc = bacc.Bacc()

    sz_h = nc.declare_dram_parameter("raw_sizes", [BC, S], F16, isOutput=False)
    dl_h = nc.declare_dram_parameter("raw_delays", [BC, S], F16, isOutput=False)
    dr_h = nc.declare_dram_parameter("raw_directions", [BC, S], F16, isOutput=False)
    v_h = nc.declare_dram_parameter("vpack", [BC, 8], F32, isOutput=False)
    out_h = nc.declare_dram_parameter("partial", [1, 1], F32, isOutput=True)

    out_dma_holder = []
    with tile.TileContext(nc) as tc, ExitStack() as ctx:
        _patch_drain(tc, out_dma_holder)
        sm = ctx.enter_context(tc.tile_pool(name="sm", bufs=1))
        scr = ctx.enter_context(tc.tile_pool(name="scr", bufs=2))
        pp = ctx.enter_context(tc.tile_pool(name="pp", bufs=1, space="PSUM"))

        # big input tensors: one SBUF tensor per input, tile t = cols [t*S,(t+1)*S)
        SZ = sm.tile([P, NT * S], F16, tag="SZ", name="SZ")
        DL = sm.tile([P, NT * S], F16, tag="DL", name="DL")
        DR = sm.tile([P, NT * S], F16, tag="DR", name="DR")
        V = sm.tile([P, NT * 8], F32, tag="V", name="V")
        Rb = sm.tile([P, NT * NR], F32, tag="Rb", name="Rb")
        W = sm.tile([P, NT * NR], F32, tag="W", name="W")

        _consts = {}

        def constv(val):
            if val not in _consts:
                cname = f"cst{len(_consts)}"
                ct = sm.tile([P, 1], F32, tag=cname, name=cname)
                nc.vector.memset(ct[:, :], val)
                _consts[val] = ct[:, :]
            return _consts[val]

        # DRAM views: tile t holds rows r = p*NT + t
        sz_t = sz_h[:, :].rearrange("(p t) s -> t p s", t=NT)
        dl_t = dl_h[:, :].rearrange("(p t) s -> t p s", t=NT)
        dr_t = dr_h[:, :].rearrange("(p t) s -> t p s", t=NT)
        v_d = v_h[:, :].rearrange("(p t) v -> p (t v)", t=NT)

        def szs(t):
            return slice(t * S, (t + 1) * S)

        # ---- DMA triggers: all on the sync HWDGE ring, arrival order =
        # compute order; tile 0 streams as halves for a fast pipeline fill.
        nc.sync.dma_start(V[:, :], v_d)
        nc.sync.dma_start(SZ[:, 0:H], sz_t[0][:, 0:H])
        nc.sync.dma_start(SZ[:, H:S], sz_t[0][:, H:S])
        nc.sync.dma_start(DR[:, 0:H], dr_t[0][:, 0:H])
        nc.sync.dma_start(DR[:, H:S], dr_t[0][:, H:S])
        nc.sync.dma_start(DL[:, szs(0)], dl_t[0])
        nc.sync.dma_start(SZ[:, szs(1)], sz_t[1])
        nc.sync.dma_start(DR[:, szs(1)], dr_t[1])
        nc.sync.dma_start(DL[:, szs(1)], dl_t[1])
        nc.sync.dma_start(SZ[:, szs(2)], sz_t[2])
        nc.sync.dma_start(DR[:, szs(2)], dr_t[2])
        nc.sync.dma_start(DL[:, szs(2)], dl_t[2])
        nc.sync.dma_start(SZ[:, szs(3)], sz_t[3])
        nc.sync.dma_start(DR[:, szs(3)], dr_t[3])
        nc.sync.dma_start(DL[:, szs(3)], dl_t[3])

        # ---- W weight tile + Rblock const columns (gpsimd memsets) ----
        # Rblock slot r semantics (per tile-column t):
        #  0: A = sum sign(sz-1400.5)        w=0.3   (0.6 * 1/2)
        #  1: B = sum sign(0.05-dl)          w=0.2   (0.4 * 1/2)
        #  2: C = sum is_equal(sz_s,sz_s-1)  w=0.2
        #  3: D = sum not_equal(dr_s,dr_s-1) w=0.1
        #  4: g1r=szlast>1400  w=-0.6    5: g1m=szmod>1400  w=+0.6
        #  6: l2r=dllast<0.05  w=-0.4    7: l2m=dlmod<0.05  w=+0.4
        #  8: e3r=szlast==szprev w=-0.2  9: e3m=|szmod-szprev|<0.5 w=+0.2
        # 10: const 1.0, w = 0.6*1024 + 0.4*1024 + 0.1   11: unused
        # 12-15: tile-0 second halves of slots 0-3 (same weights)
        Wr = W[:, :].rearrange("p (t r) -> r p t", r=NR)
        Rr = Rb[:, :].rearrange("p (t r) -> r p t", r=NR)
        g = nc.gpsimd
        g.memset(Wr[0], 0.3)
        g.memset(Wr[1], 0.2)
        g.memset(Wr[2], 0.2)
        g.memset(Wr[3], 0.1)
        g.memset(Wr[4], -0.6)
        g.memset(Wr[5], 0.6)
        g.memset(Wr[6], -0.4)
        g.memset(Wr[7], 0.4)
        g.memset(Wr[8], -0.2)
        g.memset(Wr[9], 0.2)
        g.memset(Wr[10], 0.6 * 1024.0 + 0.4 * 1024.0 + 0.1)
        g.memset(Wr[11], 0.0)
        g.memset(Rr[10], 1.0)
        g.memset(Rr[11], 0.0)
        g.memset(Wr[12][:, 0:1], 0.3)
        g.memset(Wr[14][:, 0:1], 0.2)
        g.memset(Wr[15][:, 0:1], 0.1)
        g.memset(Wr[12][:, 1:4], 0.0)
        g.memset(Wr[13], 0.0)
        g.memset(Wr[14][:, 1:4], 0.0)
        g.memset(Wr[15][:, 1:4], 0.0)
        for r in range(12, 16):
            g.memset(Rr[r], 0.0)

        v = nc.vector

        def rslot(t, r):
            c = t * NR + r
            return Rb[:, c:c + 1]

        # ---- fused count ops (accum_out -> Rblock) ----
        def act_sign_sz(cs, slot):
            o = scr.tile([P, cs.stop - cs.start], F16, tag="osg")
            nc.scalar.activation(o[:, :], SZ[:, cs], ACTF.Sign,
                                 bias=constv(-1400.5), scale=1.0,
                                 accum_out=slot)

        def act_sign_dl(cs, slot):
            o = scr.tile([P, cs.stop - cs.start], F16, tag="osg")
            nc.scalar.activation(o[:, :], DL[:, cs], ACTF.Sign,
                                 bias=constv(0.05), scale=-1.0,
                                 accum_out=slot)

        def dve_eq_sz(lo, hi, slot):
            o = scr.tile([P, hi - lo], F16, tag="oeq")
            v.scalar_tensor_tensor(
                o[:, :], SZ[:, lo + 1:hi + 1], 0.0, SZ[:, lo:hi],
                ALU.bypass, ALU.is_equal, accum_out=slot)

        def dve_ne_dr(lo, hi, slot):
            o = scr.tile([P, hi - lo], F16, tag="one")
            v.scalar_tensor_tensor(
                o[:, :], DR[:, lo + 1:hi + 1], 0.0, DR[:, lo:hi],
                ALU.bypass, ALU.not_equal, accum_out=slot)

        # scalar engine queue, in data-arrival order.  The dummy Sign on a
        # const forces the ACT table load at kernel start (no data deps)
        # instead of gating the first real ACTIVATE.
        dmy = sm.tile([P, 1], F16, tag="dmy", name="dmy")
        nc.scalar.activation(dmy[:, :], constv(0.0), ACTF.Sign,
                             bias=constv(-1400.5), scale=1.0)
        act_sign_sz(slice(0, H), rslot(0, 0))
        act_sign_sz(slice(H, S), rslot(0, 12))
        act_sign_dl(slice(0, S), rslot(0, 1))
        act_sign_sz(slice(S, 2 * S), rslot(1, 0))
        act_sign_dl(slice(S, 2 * S), rslot(1, 1))
        act_sign_sz(slice(2 * S, 3 * S), rslot(2, 0))
        act_sign_dl(slice(2 * S, 3 * S), rslot(2, 1))
        act_sign_sz(slice(3 * S, 4 * S), rslot(3, 0))
        act_sign_dl(slice(3 * S, 4 * S), rslot(3, 1))

        # vector engine queue, in data-arrival order
        dve_eq_sz(0, H - 1, rslot(0, 2))            # pairs s=1..H-1
        dve_eq_sz(H - 1, S - 1, rslot(0, 14))       # pairs s=H..S-1
        dve_ne_dr(0, H - 1, rslot(0, 3))
        dve_ne_dr(H - 1, S - 1, rslot(0, 15))
        dve_eq_sz(S, 2 * S - 1, rslot(1, 2))
        dve_ne_dr(S, 2 * S - 1, rslot(1, 3))
        dve_eq_sz(2 * S, 3 * S - 1, rslot(2, 2))
        dve_ne_dr(2 * S, 3 * S - 1, rslot(2, 3))
        dve_eq_sz(3 * S, 4 * S - 1, rslot(3, 2))
        dve_ne_dr(3 * S, 4 * S - 1, rslot(3, 3))

        # ---- per-sample fixups ([128, NT] strided views) ----
        SZr = SZ[:, :].rearrange("p (t s) -> s p t", s=S)
        DLr = DL[:, :].rearrange("p (t s) -> s p t", s=S)
        Vr = V[:, :].rearrange("p (t v) -> v p t", v=8)
        szlast, szprev, dllast = SZr[S - 1], SZr[S - 2], DLr[S - 1]
        padxv, dladdv, mltv, e0v, w1v = Vr[0], Vr[1], Vr[2], Vr[3], Vr[4]

        szmod = sm.tile([P, NT], F32, tag="szmod", name="szmod")
        v.tensor_tensor(szmod[:, :], szlast, padxv, ALU.add)
        v.tensor_scalar(szmod[:, :], szmod[:, :], 1500.0, None, ALU.min)
        dlmod = sm.tile([P, NT], F32, tag="dlmod", name="dlmod")
        v.tensor_tensor(dlmod[:, :], dllast, dladdv, ALU.add)

        v.tensor_scalar(Rr[4], szlast, 1400.0, None, ALU.is_gt)
        v.tensor_scalar(Rr[5], szmod[:, :], 1400.0, None, ALU.is_gt)
        v.tensor_scalar(Rr[6], dllast, 0.05, None, ALU.is_lt)
        v.tensor_scalar(Rr[7], dlmod[:, :], 0.05, None, ALU.is_lt)
        v.tensor_tensor(Rr[8], szlast, szprev, ALU.is_equal)
        d3 = sm.tile([P, NT], F32, tag="d3", name="d3")
        v.tensor_tensor(d3[:, :], szmod[:, :], szprev, ALU.subtract)
        a3 = sm.tile([P, NT], F32, tag="a3", name="a3")
        nc.scalar.activation(a3[:, :], d3[:, :], ACTF.Abs)
        v.tensor_scalar(Rr[9], a3[:, :], 0.5, None, ALU.is_lt)

        # ---- merge: scores per sample, then loss terms ----
        M = sm.tile([P, NT * NR], F32, tag="M", name="M")
        v.tensor_tensor(M[:, :], Rb[:, :], W[:, :], ALU.mult)
        s0 = sm.tile([P, NT], F32, tag="s0", name="s0")
        v.tensor_reduce(s0[:, :], M[:, :].rearrange("p (t r) -> p t r", r=NR),
                        axis=mybir.AxisListType.X, op=ALU.add)
        scores = sm.tile([P, NT], F32, tag="scores", name="scores")
        v.tensor_tensor(scores[:, :], s0[:, :], mltv, ALU.mult)

        ev = sm.tile([P, NT], F32, tag="ev", name="ev")
        v.tensor_scalar(ev[:, :], scores[:, :], 30.0, None, ALU.is_lt)
        dpi = sm.tile([P, NT], F32, tag="dpi", name="dpi")
        v.tensor_scalar(dpi[:, :], scores[:, :], 15.0, -15.0, ALU.max, ALU.add)

        u1 = sm.tile([P, NT], F32, tag="u1", name="u1")
        v.tensor_tensor(u1[:, :], ev[:, :], w1v, ALU.mult)
        v.tensor_tensor(u1[:, :], u1[:, :], e0v, ALU.add)
        ctot = sm.tile([P, NT], F32, tag="ctot", name="ctot")
        v.scalar_tensor_tensor(ctot[:, :], dpi[:, :], 2.0 / 30.0, u1[:, :],
                               ALU.mult, ALU.add)

        red = sm.tile([P, 1], F32, tag="red", name="red")
        v.tensor_reduce(red[:, :], ctot[:, :], axis=mybir.AxisListType.X,
                        op=ALU.add)

        # partition reduction on the (idle) tensor engine -> [1,1] PSUM
        ps = pp.tile([1, 1], F32, tag="ps", name="ps")
        nc.tensor.matmul(ps[:, :], red[:, :], constv(1.0), start=True, stop=True)
        outsb = sm.tile([1, 1], F32, tag="outsb", name="outsb")
        v.tensor_copy(outsb[:, :], ps[:, :])
        out_dma_holder.append(nc.sync.dma_start(out_h[:, :], outsb[:, :]))

    nc.finalize()
    return nc


def _get_nc() -> bass.Bass:
    global _NC_CACHE
    if _NC_CACHE is None:
        _NC_CACHE = _build_nc()
    return _NC_CACHE


def kernel(raw_sizes, raw_delays, raw_directions, delay_ms, padding_norm,
           confidence, profile_ids, trace=False, tmpdir=None):
    global LAST_RESULTS
    sz16 = np.asarray(raw_sizes).astype(np.float16)
    dl16 = np.asarray(raw_delays, dtype=np.float32).astype(np.float16)
    dr16 = np.asarray(raw_directions).astype(np.float16)
    dms = np.asarray(delay_ms, dtype=np.float32)
    pad = np.asarray(padding_norm, dtype=np.float32)
    conf = np.asarray(confidence, dtype=np.float32)
    pid = np.asarray(profile_ids).astype(np.int64)

    td = TARGET_DELAY[pid]
    tp = TARGET_PAD[pid]
    mult = CONFIG_MULT[pid % 4]
    sim = np.abs(dms - td) + np.abs(pad - tp)
    eff = np.maximum(dms - 20.0, 0.0) / 20.0 + np.maximum(pad - 0.3, 0.0)
    vpack = np.zeros((B, 8), dtype=np.float32)
    vpack[:, 0] = pad * 1500.0
    vpack[:, 1] = dms
    vpack[:, 2] = mult * (100.0 / S)
    vpack[:, 3] = 0.5 * sim + 0.3 * eff + 0.2 * conf * conf
    vpack[:, 4] = 0.2 * (1.0 - 2.0 * conf)

    nc = _get_nc()
    in_maps = []
    for i in range(N_CORES):
        r = slice(i * BC, (i + 1) * BC)
        in_maps.append({
            "raw_sizes": sz16[r],
            "raw_delays": dl16[r],
            "raw_directions": dr16[r],
            "vpack": vpack[r],
        })

    LAST_RESULTS = run_bass_kernel_spmd(nc, in_maps, list(range(N_CORES)),
                                        trace=trace, tmpdir=tmpdir)
    partials = [LAST_RESULTS.results[i]["partial"] for i in range(N_CORES)]
    total = float(np.sum(np.stack(partials), dtype=np.float64))
    return np.float32(total / B)


# revision 14
# speedup vs baseline: 1.7415x; 1.0155x over previous
"""AdversarialMorphingLoss — Trainium2 Bass kernel (8-core data parallel).

Full inputs arrive on the host; we shard the batch dim (B=4096) into 8
contiguous blocks of 512 rows, run one SPMD Bass program on all 8
NeuronCores, and each core returns the partial (un-normalized) sum of the
per-sample loss contribution over its 512 rows.  The host sums the 8
partials and divides by B.

Host-side prep (O(B) math + dtype casts):
  * the three [B, S] trace tensors are cast to fp16 (sizes/directions are
    exact; delays lose ~1e-4 relative — loss delta ~2e-7, tol 2e-2).
    Halves HBM traffic per core from 12.6 MB to 6.05 MB.
  * per-sample quantities depending only on [B] vectors are folded into a
    packed [B, 8] f32 tensor V:
      V0 = padding_norm*1500, V1 = delay_ms, V2 = CONFIG_MULT[pid%4]*100/S,
      V3 = 0.5*sim + 0.3*eff + 0.2*conf^2, V4 = 0.2*(1-2*conf)
    so ctot_b = (2/30)*relu(scores-15) + V3 + V4*(scores<30).

Device strategy (measured op rates on this toolchain):
  * DVE scalar_tensor_tensor w/ accum_out: 1x (2.29us per 2048-col tile)
    — the only fused compare+row-reduce DVE form that works on HW.
  * ScalarE ACTIVATE w/ accum_out: 1x @1.2GHz (2.0us) — Sign-based
    threshold counts.  16 fused count ops total, split 8/8:
      ACT: (sz>1400) and (dl<0.05) via Sign (sign-sum convention)
      DVE: sz[s]==sz[s-1] (is_equal), dir[s]!=dir[s-1] (not_equal)
  * all 16 input DMA triggers issue from the sync queue; tile 0's three
    tensors stream as 256KB halves so compute starts ~5us earlier.
  * every accumulator + last-col fixup lands in a [128, 64] f32 Rblock
    (16 slots x 4 tiles; slots 12-15 hold tile-0's second halves); the
    whole weighted merge is one tensor_tensor against a memset W tile +
    one strided tensor_reduce.
  * partition reduction via TensorE matmul into PSUM so the output DMA
    is one 4-byte descriptor (a [128,1] scatter costs ~4us completion).
"""

import numpy as np
from contextlib import ExitStack

import concourse.bass as bass
import concourse.bacc as bacc
import concourse.mybir as mybir
from concourse import tile
from concourse.bass_utils import run_bass_kernel_spmd

B, S = 4096, 2048
N_CORES = 8
BC = B // N_CORES          # 512 rows per core
P = 128                    # SBUF partitions
NT = BC // P               # 4 tiles of 128 rows per core
NR = 16                    # Rblock slots per tile
H = S // 2

F32 = mybir.dt.float32
F16 = mybir.dt.float16
ALU = mybir.AluOpType
ACTF = mybir.ActivationFunctionType

# per-profile targets (match reference.py)
TARGET_DELAY = np.array([2.0, 1.0, 0.5, 5.0, 3.0], dtype=np.float32)
TARGET_PAD = np.array([0.08, 0.12, 0.05, 0.15, 0.10], dtype=np.float32)
CONFIG_MULT = np.array([1.0, 1.3, 1.6, 2.0], dtype=np.float32)

_NC_CACHE = None
LAST_RESULTS = None        # BassKernelResults of the last kernel() call


def _patch_drain(tc, out_dma_holder):
    """Slim TileContext's exit drain (controlled by KERNEL_DRAIN_MODE):
    'full'     stock ending (drain + EVSEM barrier + sem clear + barrier)
    'nobar2'   stock minus the trailing all-engine barrier
    'plainsem' plain-semaphore ending (see baseline notes).
    """
    import os
    import re
    import types
    from concourse.vector_clock import ScopedClock

    mode = os.environ.get("KERNEL_DRAIN_MODE", "nobar2")
    if mode == "full":
        return

    def _slim(self, tick_clock, wait_clock):
        nc = self.nc
        if mode == "plainsem":
            totals = {}
            upd_re = re.compile(r"update:S\[([A-Za-z0-9_]+)\](?:\+\+|\+=)(\d+)")
            for bb in nc.main_func.blocks:
                for ins in bb.instructions:
                    for mm in upd_re.finditer(str(ins)):
                        totals[mm.group(1)] = totals.get(mm.group(1), 0) + int(mm.group(2))
            by_name = {h.name: h for h in self.sems.allocated().values()}
            waits = [(h, totals[name]) for name, h in sorted(by_name.items())
                     if totals.get(name, 0) > 0]
            for eng in nc.engines.values():
                for h, total in waits:
                    eng.wait_ge(h, total)
            popped = nc._tile_sem_poison_stack.pop()
            assert popped is self._sem_poison
            nc.clear_and_free_semaphores(
                list(self.sems.allocated().values()))
            return
        drain_inst = nc.sync.drain()
        wait_clock.add_sem_waits(
            drain_inst.ins, ScopedClock({None: tick_clock.global_clock}))
        nc.all_engine_barrier()
        popped = nc._tile_sem_poison_stack.pop()
        assert popped is self._sem_poison
        nc.clear_and_free_semaphores(list(self.sems.allocated().values()))

    tc._drain_and_barrier = types.MethodType(_slim, tc)


def _build_nc() -> bass.Bass:
    nc = bacc.Bacc()

    sz_h = nc.declare_dram_parameter("raw_sizes", [BC, S], F16, isOutput=False)
    dl_h = nc.declare_dram_parameter("raw_delays", [BC, S], F16, isOutput=False)
    dr_h = nc.declare_dram_parameter("raw_directions", [BC, S], F16, isOutput=False)
    v_h = nc.declare_dram_parameter("vpack", [BC, 8], F32, isOutput=False)
    out_h = nc.declare_dram_parameter("partial", [1, 1], F32, isOutput=True)

    out_dma_holder = []
    with tile.TileContext(nc) as tc, ExitStack() as ctx:
        _patch_drain(tc, out_dma_holder)
        sm = ctx.enter_context(tc.tile_pool(name="sm", bufs=1))
        scr = ctx.enter_context(tc.tile_pool(name="scr", bufs=2))
        pp = ctx.enter_context(tc.tile_pool(name="pp", bufs=1, space="PSUM"))

        # big input tensors: one SBUF tensor per input, tile t = cols [t*S,(t+1)*S)
        SZ = sm.tile([P, NT * S], F16, tag="SZ", name="SZ")
        DL = sm.tile([P, NT * S], F16, tag="DL", name="DL")
        DR = sm.tile([P, NT * S], F16, tag="DR", name="DR")
        V = sm.tile([P, NT * 8], F32, tag="V", name="V")
        Rb = sm.tile([P, NT * NR], F32, tag="Rb", name="Rb")
        W = sm.tile([P, NT * NR], F32, tag="W", name="W")

        _consts = {}

        def constv(val):
            if val not in _consts:
                cname = f"cst{len(_consts)}"
                ct = sm.tile([P, 1], F32, tag=cname, name=cname)
                nc.vector.memset(ct[:, :], val)
                _consts[val] = ct[:, :]
            return _consts[val]

        # DRAM views: tile t holds rows r = p*NT + t
        sz_t = sz_h[:, :].rearrange("(p t) s -> t p s", t=NT)
        dl_t = dl_h[:, :].rearrange("(p t) s -> t p s", t=NT)
        dr_t = dr_h[:, :].rearrange("(p t) s -> t p s", t=NT)
        v_d = v_h[:, :].rearrange("(p t) v -> p (t v)", t=NT)

        def szs(t):
            return slice(t * S, (t + 1) * S)

        # ---- DMA triggers: all on the sync HWDGE ring, arrival order =
        # compute order; tile 0 streams as halves for a fast pipeline fill.
        nc.sync.dma_start(V[:, :], v_d)
        nc.sync.dma_start(SZ[:, 0:H], sz_t[0][:, 0:H])
        nc.sync.dma_start(SZ[:, H:S], sz_t[0][:, H:S])
        nc.sync.dma_start(DR[:, 0:H], dr_t[0][:, 0:H])
        nc.sync.dma_start(DR[:, H:S], dr_t[0][:, H:S])
        nc.sync.dma_start(SZ[:, szs(1)], sz_t[1])
        nc.sync.dma_start(DL[:, szs(0)], dl_t[0])
        nc.sync.dma_start(DR[:, szs(1)], dr_t[1])
        nc.sync.dma_start(DL[:, szs(1)], dl_t[1])
        nc.sync.dma_start(SZ[:, szs(2)], sz_t[2])
        nc.sync.dma_start(DR[:, szs(2)], dr_t[2])
        nc.sync.dma_start(DL[:, szs(2)], dl_t[2])
        nc.sync.dma_start(SZ[:, szs(3)], sz_t[3])
        nc.sync.dma_start(DR[:, szs(3)], dr_t[3])
        nc.sync.dma_start(DL[:, szs(3)], dl_t[3])

        # ---- W weight tile + Rblock const columns (gpsimd memsets) ----
        # Rblock slot r semantics (per tile-column t):
        #  0: A = sum sign(sz-1400.5)        w=0.3   (0.6 * 1/2)
        #  1: B = sum sign(0.05-dl)          w=0.2   (0.4 * 1/2)
        #  2: C = sum is_equal(sz_s,sz_s-1)  w=0.2
        #  3: D = sum not_equal(dr_s,dr_s-1) w=0.1
        #  4: g1r=szlast>1400  w=-0.6    5: g1m=szmod>1400  w=+0.6
        #  6: l2r=dllast<0.05  w=-0.4    7: l2m=dlmod<0.05  w=+0.4
        #  8: e3r=szlast==szprev w=-0.2  9: e3m=|szmod-szprev|<0.5 w=+0.2
        # 10: const 1.0, w = 0.6*1024 + 0.4*1024 + 0.1   11: unused
        # 12-15: tile-0 second halves of slots 0-3 (same weights)
        Wr = W[:, :].rearrange("p (t r) -> r p t", r=NR)
        Rr = Rb[:, :].rearrange("p (t r) -> r p t", r=NR)
        g = nc.gpsimd
        g.memset(Wr[0], 0.3)
        g.memset(Wr[1], 0.2)
        g.memset(Wr[2], 0.2)
        g.memset(Wr[3], 0.1)
        g.memset(Wr[4], -0.3)
        g.memset(Wr[5], 0.6)
        g.memset(Wr[6], -0.2)
        g.memset(Wr[7], 0.4)
        g.memset(Wr[8], -0.2)
        g.memset(Wr[9], 0.2)
        g.memset(Wr[10], 0.6 * 1024.0 + 0.4 * 1024.0 + 0.1 - 0.3 - 0.2)
        g.memset(Wr[11], 0.0)
        g.memset(Rr[10], 1.0)
        g.memset(Rr[11], 0.0)
        g.memset(Wr[12][:, 0:1], 0.3)
        g.memset(Wr[14][:, 0:1], 0.2)
        g.memset(Wr[15][:, 0:1], 0.1)
        g.memset(Wr[12][:, 1:4], 0.0)
        g.memset(Wr[13], 0.0)
        g.memset(Wr[14][:, 1:4], 0.0)
        g.memset(Wr[15][:, 1:4], 0.0)
        for r in range(12, 16):
            g.memset(Rr[r], 0.0)

        v = nc.vector

        def rslot(t, r):
            c = t * NR + r
            return Rb[:, c:c + 1]

        # ---- fused count ops (accum_out -> Rblock) ----
        def act_sign_sz(cs, slot):
            o = scr.tile([P, cs.stop - cs.start], F16, tag="osg")
            nc.scalar.activation(o[:, :], SZ[:, cs], ACTF.Sign,
                                 bias=constv(-1400.5), scale=1.0,
                                 accum_out=slot)

        def act_sign_dl(cs, slot):
            o = scr.tile([P, cs.stop - cs.start], F16, tag="osg")
            nc.scalar.activation(o[:, :], DL[:, cs], ACTF.Sign,
                                 bias=constv(0.05), scale=-1.0,
                                 accum_out=slot)

        def dve_eq_sz(lo, hi, slot):
            o = scr.tile([P, hi - lo], F16, tag="oeq")
            v.scalar_tensor_tensor(
                o[:, :], SZ[:, lo + 1:hi + 1], 0.0, SZ[:, lo:hi],
                ALU.bypass, ALU.is_equal, accum_out=slot)

        def dve_ne_dr(lo, hi, slot):
            o = scr.tile([P, hi - lo], F16, tag="one")
            v.scalar_tensor_tensor(
                o[:, :], DR[:, lo + 1:hi + 1], 0.0, DR[:, lo:hi],
                ALU.bypass, ALU.not_equal, accum_out=slot)

        # scalar engine queue, in data-arrival order.  The dummy Sign on a
        # const forces the ACT table load at kernel start (no data deps)
        # instead of gating the first real ACTIVATE.
        dmy = sm.tile([P, 1], F16, tag="dmy", name="dmy")
        nc.scalar.activation(dmy[:, :], constv(0.0), ACTF.Sign,
                             bias=constv(-1400.5), scale=1.0)
        act_sign_sz(slice(0, H), rslot(0, 0))
        act_sign_sz(slice(H, S), rslot(0, 12))
        act_sign_sz(slice(S, 2 * S), rslot(1, 0))
        act_sign_dl(slice(0, S), rslot(0, 1))
        act_sign_dl(slice(S, 2 * S), rslot(1, 1))
        act_sign_sz(slice(2 * S, 3 * S), rslot(2, 0))
        act_sign_dl(slice(2 * S, 3 * S), rslot(2, 1))
        act_sign_sz(slice(3 * S, 4 * S), rslot(3, 0))
        act_sign_dl(slice(3 * S, 4 * S), rslot(3, 1))

        # vector engine queue, in data-arrival order
        dve_eq_sz(0, H - 1, rslot(0, 2))            # pairs s=1..H-1
        dve_eq_sz(H - 1, S - 1, rslot(0, 14))       # pairs s=H..S-1
        dve_ne_dr(0, H - 1, rslot(0, 3))
        dve_ne_dr(H - 1, S - 1, rslot(0, 15))
        dve_eq_sz(S, 2 * S - 1, rslot(1, 2))
        dve_ne_dr(S, 2 * S - 1, rslot(1, 3))
        dve_eq_sz(2 * S, 3 * S - 1, rslot(2, 2))
        dve_ne_dr(2 * S, 3 * S - 1, rslot(2, 3))
        dve_eq_sz(3 * S, 4 * S - 1, rslot(3, 2))
        dve_ne_dr(3 * S, 4 * S - 1, rslot(3, 3))

        # ---- per-sample fixups ([128, NT] strided views) ----
        SZr = SZ[:, :].rearrange("p (t s) -> s p t", s=S)
        DLr = DL[:, :].rearrange("p (t s) -> s p t", s=S)
        Vr = V[:, :].rearrange("p (t v) -> v p t", v=8)
        szlast, szprev, dllast = SZr[S - 1], SZr[S - 2], DLr[S - 1]
        padxv, dladdv, mltv, e0v, w1v = Vr[0], Vr[1], Vr[2], Vr[3], Vr[4]

        szmod = sm.tile([P, NT], F32, tag="szmod", name="szmod")
        v.tensor_tensor(szmod[:, :], szlast, padxv, ALU.add)
        v.tensor_scalar(szmod[:, :], szmod[:, :], 1500.0, None, ALU.min)
        dlmod = sm.tile([P, NT], F32, tag="dlmod", name="dlmod")
        v.tensor_tensor(dlmod[:, :], dllast, dladdv, ALU.add)

        # g1r/l2r run on ScalarE as Sign forms (w/const adjusted: slot4 holds
        # sign(szlast-1400.5) with w=-0.3, slot6 sign(0.05-dllast) w=-0.2,
        # const -0.5 folded into slot 10's weight)
        nc.scalar.activation(Rr[4], szlast, ACTF.Sign,
                             bias=constv(-1400.5), scale=1.0)
        nc.scalar.activation(Rr[6], dllast, ACTF.Sign,
                             bias=constv(0.05), scale=-1.0)
        v.tensor_scalar(Rr[5], szmod[:, :], 1400.0, None, ALU.is_gt)
        v.tensor_scalar(Rr[7], dlmod[:, :], 0.05, None, ALU.is_lt)
        v.tensor_tensor(Rr[8], szlast, szprev, ALU.is_equal)
        d3 = sm.tile([P, NT], F32, tag="d3", name="d3")
        v.tensor_tensor(d3[:, :], szmod[:, :], szprev, ALU.subtract)
        v.tensor_tensor(d3[:, :], d3[:, :], d3[:, :], ALU.mult)
        v.tensor_scalar(Rr[9], d3[:, :], 0.25, None, ALU.is_lt)

        # ---- merge: scores per sample, then loss terms ----
        M = sm.tile([P, NT * NR], F32, tag="M", name="M")
        v.tensor_tensor(M[:, :], Rb[:, :], W[:, :], ALU.mult)
        s0 = sm.tile([P, NT], F32, tag="s0", name="s0")
        v.tensor_reduce(s0[:, :], M[:, :].rearrange("p (t r) -> p t r", r=NR),
                        axis=mybir.AxisListType.X, op=ALU.add)
        scores = sm.tile([P, NT], F32, tag="scores", name="scores")
        v.tensor_tensor(scores[:, :], s0[:, :], mltv, ALU.mult)

        ev = sm.tile([P, NT], F32, tag="ev", name="ev")
        v.tensor_scalar(ev[:, :], scores[:, :], 30.0, None, ALU.is_lt)
        dpi = sm.tile([P, NT], F32, tag="dpi", name="dpi")
        v.tensor_scalar(dpi[:, :], scores[:, :], 15.0, -15.0, ALU.max, ALU.add)

        u1 = sm.tile([P, NT], F32, tag="u1", name="u1")
        v.tensor_tensor(u1[:, :], ev[:, :], w1v, ALU.mult)
        v.tensor_tensor(u1[:, :], u1[:, :], e0v, ALU.add)
        ctot = sm.tile([P, NT], F32, tag="ctot", name="ctot")
        v.scalar_tensor_tensor(ctot[:, :], dpi[:, :], 2.0 / 30.0, u1[:, :],
                               ALU.mult, ALU.add)

        red = sm.tile([P, 1], F32, tag="red", name="red")
        v.tensor_reduce(red[:, :], ctot[:, :], axis=mybir.AxisListType.X,
                        op=ALU.add)

        # partition reduction on the (idle) tensor engine -> [1,1] PSUM
        ps = pp.tile([1, 1], F32, tag="ps", name="ps")
        nc.tensor.matmul(ps[:, :], red[:, :], constv(1.0), start=True, stop=True)
        outsb = sm.tile([1, 1], F32, tag="outsb", name="outsb")
        v.tensor_copy(outsb[:, :], ps[:, :])
        out_dma_holder.append(nc.sync.dma_start(out_h[:, :], outsb[:, :]))

    nc.finalize()
    return nc


def _get_nc() -> bass.Bass:
    global _NC_CACHE
    if _NC_CACHE is None:
        _NC_CACHE = _build_nc()
    return _NC_CACHE


def kernel(raw_sizes, raw_delays, raw_directions, delay_ms, padding_norm,
           confidence, profile_ids, trace=False, tmpdir=None):
    global LAST_RESULTS
    sz16 = np.asarray(raw_sizes).astype(np.float16)
    dl16 = np.asarray(raw_delays, dtype=np.float32).astype(np.float16)
    dr16 = np.asarray(raw_directions).astype(np.float16)
    dms = np.asarray(delay_ms, dtype=np.float32)
    pad = np.asarray(padding_norm, dtype=np.float32)
    conf = np.asarray(confidence, dtype=np.float32)
    pid = np.asarray(profile_ids).astype(np.int64)

    td = TARGET_DELAY[pid]
    tp = TARGET_PAD[pid]
    mult = CONFIG_MULT[pid % 4]
    sim = np.abs(dms - td) + np.abs(pad - tp)
    eff = np.maximum(dms - 20.0, 0.0) / 20.0 + np.maximum(pad - 0.3, 0.0)
    vpack = np.zeros((B, 8), dtype=np.float32)
    vpack[:, 0] = pad * 1500.0
    vpack[:, 1] = dms
    vpack[:, 2] = mult * (100.0 / S)
    vpack[:, 3] = 0.5 * sim + 0.3 * eff + 0.2 * conf * conf
    vpack[:, 4] = 0.2 * (1.0 - 2.0 * conf)

    nc = _get_nc()
    in_maps = []
    for i in range(N_CORES):
        r = slice(i * BC, (i + 1) * BC)
        in_maps.append({
            "raw_sizes": sz16[r],
            "raw_delays": dl16[r],
            "raw_directions": dr16[r],
            "vpack": vpack[r],
        })

    LAST_RESULTS = run_bass_kernel_spmd(nc, in_maps, list(range(N_CORES)),
                                        trace=trace, tmpdir=tmpdir)
    partials = [LAST_RESULTS.results[i]["partial"] for i in range(N_CORES)]
    total = float(np.sum(np.stack(partials), dtype=np.float64))
    return np.float32(total / B)


# revision 15
# speedup vs baseline: 1.7762x; 1.0199x over previous
"""AdversarialMorphingLoss — Trainium2 Bass kernel (8-core data parallel).

Full inputs arrive on the host; we shard the batch dim (B=4096) into 8
contiguous blocks of 512 rows, run one SPMD Bass program on all 8
NeuronCores, and each core returns the partial (un-normalized) sum of the
per-sample loss contribution over its 512 rows.  The host sums the 8
partials and divides by B.

Host-side prep (O(B) math + dtype casts):
  * the three [B, S] trace tensors are cast to fp16 (sizes/directions are
    exact; delays lose ~1e-4 relative — loss delta ~2e-7, tol 2e-2).
    Halves HBM traffic per core from 12.6 MB to 6.05 MB.
  * per-sample quantities depending only on [B] vectors are folded into a
    packed [B, 8] f32 tensor V:
      V0 = padding_norm*1500, V1 = delay_ms, V2 = CONFIG_MULT[pid%4]*100/S,
      V3 = 0.5*sim + 0.3*eff + 0.2*conf^2, V4 = 0.2*(1-2*conf)
    so ctot_b = (2/30)*relu(scores-15) + V3 + V4*(scores<30).

Device strategy (measured op rates on this toolchain):
  * DVE scalar_tensor_tensor w/ accum_out: 1x (2.29us per 2048-col tile)
    — the only fused compare+row-reduce DVE form that works on HW.
  * ScalarE ACTIVATE w/ accum_out: 1x @1.2GHz (2.0us) — Sign-based
    threshold counts.  16 fused count ops total, split 8/8:
      ACT: (sz>1400) and (dl<0.05) via Sign (sign-sum convention)
      DVE: sz[s]==sz[s-1] (is_equal), dir[s]!=dir[s-1] (not_equal)
  * all 16 input DMA triggers issue from the sync queue; tile 0's three
    tensors stream as 256KB halves so compute starts ~5us earlier.
  * every accumulator + last-col fixup lands in a [128, 64] f32 Rblock
    (16 slots x 4 tiles; slots 12-15 hold tile-0's second halves); the
    whole weighted merge is one tensor_tensor against a memset W tile +
    one strided tensor_reduce.
  * partition reduction via TensorE matmul into PSUM so the output DMA
    is one 4-byte descriptor (a [128,1] scatter costs ~4us completion).
"""

import numpy as np
from contextlib import ExitStack

import concourse.bass as bass
import concourse.bacc as bacc
import concourse.mybir as mybir
from concourse import tile
from concourse.bass_utils import run_bass_kernel_spmd

B, S = 4096, 2048
N_CORES = 8
BC = B // N_CORES          # 512 rows per core
P = 128                    # SBUF partitions
NT = BC // P               # 4 tiles of 128 rows per core
NR = 16                    # Rblock slots per tile
H = S // 2

F32 = mybir.dt.float32
F16 = mybir.dt.float16
U8 = mybir.dt.uint8
ALU = mybir.AluOpType
ACTF = mybir.ActivationFunctionType

# per-profile targets (match reference.py)
TARGET_DELAY = np.array([2.0, 1.0, 0.5, 5.0, 3.0], dtype=np.float32)
TARGET_PAD = np.array([0.08, 0.12, 0.05, 0.15, 0.10], dtype=np.float32)
CONFIG_MULT = np.array([1.0, 1.3, 1.6, 2.0], dtype=np.float32)

_NC_CACHE = None
LAST_RESULTS = None        # BassKernelResults of the last kernel() call


def _patch_drain(tc, out_dma_holder):
    """Slim TileContext's exit drain (controlled by KERNEL_DRAIN_MODE):
    'full'     stock ending (drain + EVSEM barrier + sem clear + barrier)
    'nobar2'   stock minus the trailing all-engine barrier
    'plainsem' plain-semaphore ending (see baseline notes).
    """
    import os
    import re
    import types
    from concourse.vector_clock import ScopedClock

    mode = os.environ.get("KERNEL_DRAIN_MODE", "nobar2")
    if mode == "full":
        return

    def _slim(self, tick_clock, wait_clock):
        nc = self.nc
        if mode == "plainsem":
            totals = {}
            upd_re = re.compile(r"update:S\[([A-Za-z0-9_]+)\](?:\+\+|\+=)(\d+)")
            for bb in nc.main_func.blocks:
                for ins in bb.instructions:
                    for mm in upd_re.finditer(str(ins)):
                        totals[mm.group(1)] = totals.get(mm.group(1), 0) + int(mm.group(2))
            by_name = {h.name: h for h in self.sems.allocated().values()}
            waits = [(h, totals[name]) for name, h in sorted(by_name.items())
                     if totals.get(name, 0) > 0]
            for eng in nc.engines.values():
                for h, total in waits:
                    eng.wait_ge(h, total)
            popped = nc._tile_sem_poison_stack.pop()
            assert popped is self._sem_poison
            nc.clear_and_free_semaphores(
                list(self.sems.allocated().values()))
            return
        drain_inst = nc.sync.drain()
        wait_clock.add_sem_waits(
            drain_inst.ins, ScopedClock({None: tick_clock.global_clock}))
        nc.all_engine_barrier()
        popped = nc._tile_sem_poison_stack.pop()
        assert popped is self._sem_poison
        nc.clear_and_free_semaphores(list(self.sems.allocated().values()))

    tc._drain_and_barrier = types.MethodType(_slim, tc)


def _build_nc() -> bass.Bass:
    nc = bacc.Bacc()

    sz_h = nc.declare_dram_parameter("raw_sizes", [BC, S], F16, isOutput=False)
    dl_h = nc.declare_dram_parameter("raw_delays", [BC, S], F16, isOutput=False)
    dr_h = nc.declare_dram_parameter("raw_directions", [BC, S], U8, isOutput=False)
    v_h = nc.declare_dram_parameter("vpack", [BC, 8], F32, isOutput=False)
    out_h = nc.declare_dram_parameter("partial", [1, 1], F32, isOutput=True)

    out_dma_holder = []
    with tile.TileContext(nc) as tc, ExitStack() as ctx:
        _patch_drain(tc, out_dma_holder)
        sm = ctx.enter_context(tc.tile_pool(name="sm", bufs=1))
        scr = ctx.enter_context(tc.tile_pool(name="scr", bufs=2))
        pp = ctx.enter_context(tc.tile_pool(name="pp", bufs=1, space="PSUM"))

        # big input tensors: one SBUF tensor per input, tile t = cols [t*S,(t+1)*S)
        SZ = sm.tile([P, NT * S], F16, tag="SZ", name="SZ")
        DL = sm.tile([P, NT * S], F16, tag="DL", name="DL")
        DR = sm.tile([P, NT * S], U8, tag="DR", name="DR")
        V = sm.tile([P, NT * 8], F32, tag="V", name="V")
        Rb = sm.tile([P, NT * NR], F32, tag="Rb", name="Rb")
        W = sm.tile([P, NT * NR], F32, tag="W", name="W")

        _consts = {}

        def constv(val):
            if val not in _consts:
                cname = f"cst{len(_consts)}"
                ct = sm.tile([P, 1], F32, tag=cname, name=cname)
                nc.vector.memset(ct[:, :], val)
                _consts[val] = ct[:, :]
            return _consts[val]

        # DRAM views: tile t holds rows r = p*NT + t
        sz_t = sz_h[:, :].rearrange("(p t) s -> t p s", t=NT)
        dl_t = dl_h[:, :].rearrange("(p t) s -> t p s", t=NT)
        dr_t = dr_h[:, :].rearrange("(p t) s -> t p s", t=NT)
        v_d = v_h[:, :].rearrange("(p t) v -> p (t v)", t=NT)

        def szs(t):
            return slice(t * S, (t + 1) * S)

        # ---- DMA triggers: all on the sync HWDGE ring, arrival order =
        # compute order; tile 0 streams as halves for a fast pipeline fill.
        nc.sync.dma_start(V[:, :], v_d)
        nc.sync.dma_start(SZ[:, 0:H], sz_t[0][:, 0:H])
        nc.sync.dma_start(SZ[:, H:S], sz_t[0][:, H:S])
        nc.sync.dma_start(DR[:, 0:H], dr_t[0][:, 0:H])
        nc.sync.dma_start(DR[:, H:S], dr_t[0][:, H:S])
        nc.sync.dma_start(SZ[:, szs(1)], sz_t[1])
        nc.sync.dma_start(DL[:, szs(0)], dl_t[0])
        nc.sync.dma_start(DR[:, szs(1)], dr_t[1])
        nc.sync.dma_start(DL[:, szs(1)], dl_t[1])
        nc.sync.dma_start(SZ[:, szs(2)], sz_t[2])
        nc.sync.dma_start(DR[:, szs(2)], dr_t[2])
        nc.sync.dma_start(DL[:, szs(2)], dl_t[2])
        nc.sync.dma_start(SZ[:, szs(3)], sz_t[3])
        nc.sync.dma_start(DR[:, szs(3)], dr_t[3])
        nc.sync.dma_start(DL[:, szs(3)], dl_t[3])

        # ---- W weight tile + Rblock const columns (gpsimd memsets) ----
        # Rblock slot r semantics (per tile-column t):
        #  0: A = sum sign(sz-1400.5)        w=0.3   (0.6 * 1/2)
        #  1: B = sum sign(0.05-dl)          w=0.2   (0.4 * 1/2)
        #  2: C = sum is_equal(sz_s,sz_s-1)  w=0.2
        #  3: D = sum not_equal(dr_s,dr_s-1) w=0.1
        #  4: g1r=szlast>1400  w=-0.6    5: g1m=szmod>1400  w=+0.6
        #  6: l2r=dllast<0.05  w=-0.4    7: l2m=dlmod<0.05  w=+0.4
        #  8: e3r=szlast==szprev w=-0.2  9: e3m=|szmod-szprev|<0.5 w=+0.2
        # 10: const 1.0, w = 0.6*1024 + 0.4*1024 + 0.1   11: unused
        # 12-15: tile-0 second halves of slots 0-3 (same weights)
        Wr = W[:, :].rearrange("p (t r) -> r p t", r=NR)
        Rr = Rb[:, :].rearrange("p (t r) -> r p t", r=NR)
        g = nc.gpsimd
        g.memset(Wr[0], 0.3)
        g.memset(Wr[1], 0.2)
        g.memset(Wr[2], 0.2)
        g.memset(Wr[3], 0.1)
        g.memset(Wr[4], -0.3)
        g.memset(Wr[5], 0.6)
        g.memset(Wr[6], -0.2)
        g.memset(Wr[7], 0.4)
        g.memset(Wr[8], -0.2)
        g.memset(Wr[9], 0.2)
        g.memset(Wr[10], 0.6 * 1024.0 + 0.4 * 1024.0 + 0.1 - 0.3 - 0.2)
        g.memset(Wr[11], 0.0)
        g.memset(Rr[10], 1.0)
        g.memset(Rr[11], 0.0)
        g.memset(Wr[12][:, 0:1], 0.3)
        g.memset(Wr[14][:, 0:1], 0.2)
        g.memset(Wr[15][:, 0:1], 0.1)
        g.memset(Wr[12][:, 1:4], 0.0)
        g.memset(Wr[13], 0.0)
        g.memset(Wr[14][:, 1:4], 0.0)
        g.memset(Wr[15][:, 1:4], 0.0)
        for r in range(12, 16):
            g.memset(Rr[r], 0.0)

        v = nc.vector

        def rslot(t, r):
            c = t * NR + r
            return Rb[:, c:c + 1]

        # ---- fused count ops (accum_out -> Rblock) ----
        def act_sign_sz(cs, slot):
            o = scr.tile([P, cs.stop - cs.start], F16, tag="osg")
            nc.scalar.activation(o[:, :], SZ[:, cs], ACTF.Sign,
                                 bias=constv(-1400.5), scale=1.0,
                                 accum_out=slot)

        def act_sign_dl(cs, slot):
            o = scr.tile([P, cs.stop - cs.start], F16, tag="osg")
            nc.scalar.activation(o[:, :], DL[:, cs], ACTF.Sign,
                                 bias=constv(0.05), scale=-1.0,
                                 accum_out=slot)

        def dve_eq_sz(lo, hi, slot):
            o = scr.tile([P, hi - lo], F16, tag="oeq")
            v.scalar_tensor_tensor(
                o[:, :], SZ[:, lo + 1:hi + 1], 0.0, SZ[:, lo:hi],
                ALU.bypass, ALU.is_equal, accum_out=slot)

        def dve_ne_dr(lo, hi, slot):
            o = scr.tile([P, hi - lo], U8, tag="one")
            v.scalar_tensor_tensor(
                o[:, :], DR[:, lo + 1:hi + 1], 0.0, DR[:, lo:hi],
                ALU.bypass, ALU.not_equal, accum_out=slot)

        # scalar engine queue, in data-arrival order.  The dummy Sign on a
        # const forces the ACT table load at kernel start (no data deps)
        # instead of gating the first real ACTIVATE.
        dmy = sm.tile([P, 1], F16, tag="dmy", name="dmy")
        nc.scalar.activation(dmy[:, :], constv(0.0), ACTF.Sign,
                             bias=constv(-1400.5), scale=1.0)
        act_sign_sz(slice(0, H), rslot(0, 0))
        act_sign_sz(slice(H, S), rslot(0, 12))
        act_sign_sz(slice(S, 2 * S), rslot(1, 0))
        act_sign_dl(slice(0, S), rslot(0, 1))
        act_sign_dl(slice(S, 2 * S), rslot(1, 1))
        act_sign_sz(slice(2 * S, 3 * S), rslot(2, 0))
        act_sign_dl(slice(2 * S, 3 * S), rslot(2, 1))
        act_sign_sz(slice(3 * S, 4 * S), rslot(3, 0))
        act_sign_dl(slice(3 * S, 4 * S), rslot(3, 1))

        # vector engine queue, in data-arrival order
        dve_eq_sz(0, H - 1, rslot(0, 2))            # pairs s=1..H-1
        dve_eq_sz(H - 1, S - 1, rslot(0, 14))       # pairs s=H..S-1
        dve_ne_dr(0, H - 1, rslot(0, 3))
        dve_ne_dr(H - 1, S - 1, rslot(0, 15))
        dve_eq_sz(S, 2 * S - 1, rslot(1, 2))
        dve_ne_dr(S, 2 * S - 1, rslot(1, 3))
        dve_eq_sz(2 * S, 3 * S - 1, rslot(2, 2))
        dve_ne_dr(2 * S, 3 * S - 1, rslot(2, 3))
        dve_eq_sz(3 * S, 4 * S - 1, rslot(3, 2))
        dve_ne_dr(3 * S, 4 * S - 1, rslot(3, 3))

        # ---- per-sample fixups ([128, NT] strided views) ----
        SZr = SZ[:, :].rearrange("p (t s) -> s p t", s=S)
        DLr = DL[:, :].rearrange("p (t s) -> s p t", s=S)
        Vr = V[:, :].rearrange("p (t v) -> v p t", v=8)
        szlast, szprev, dllast = SZr[S - 1], SZr[S - 2], DLr[S - 1]
        padxv, dladdv, mltv, e0v, w1v = Vr[0], Vr[1], Vr[2], Vr[3], Vr[4]

        szmod = sm.tile([P, NT], F32, tag="szmod", name="szmod")
        v.tensor_tensor(szmod[:, :], szlast, padxv, ALU.add)
        v.tensor_scalar(szmod[:, :], szmod[:, :], 1500.0, None, ALU.min)
        dlmod = sm.tile([P, NT], F32, tag="dlmod", name="dlmod")
        v.tensor_tensor(dlmod[:, :], dllast, dladdv, ALU.add)

        # g1r/l2r run on ScalarE as Sign forms (w/const adjusted: slot4 holds
        # sign(szlast-1400.5) with w=-0.3, slot6 sign(0.05-dllast) w=-0.2,
        # const -0.5 folded into slot 10's weight)
        nc.scalar.activation(Rr[4], szlast, ACTF.Sign,
                             bias=constv(-1400.5), scale=1.0)
        nc.scalar.activation(Rr[6], dllast, ACTF.Sign,
                             bias=constv(0.05), scale=-1.0)
        v.tensor_scalar(Rr[5], szmod[:, :], 1400.0, None, ALU.is_gt)
        v.tensor_scalar(Rr[7], dlmod[:, :], 0.05, None, ALU.is_lt)
        v.tensor_tensor(Rr[8], szlast, szprev, ALU.is_equal)
        d3 = sm.tile([P, NT], F32, tag="d3", name="d3")
        v.tensor_tensor(d3[:, :], szmod[:, :], szprev, ALU.subtract)
        v.tensor_tensor(d3[:, :], d3[:, :], d3[:, :], ALU.mult)
        v.tensor_scalar(Rr[9], d3[:, :], 0.25, None, ALU.is_lt)

        # ---- merge: scores per sample, then loss terms ----
        M = sm.tile([P, NT * NR], F32, tag="M", name="M")
        v.tensor_tensor(M[:, :], Rb[:, :], W[:, :], ALU.mult)
        s0 = sm.tile([P, NT], F32, tag="s0", name="s0")
        v.tensor_reduce(s0[:, :], M[:, :].rearrange("p (t r) -> p t r", r=NR),
                        axis=mybir.AxisListType.X, op=ALU.add)
        scores = sm.tile([P, NT], F32, tag="scores", name="scores")
        v.tensor_tensor(scores[:, :], s0[:, :], mltv, ALU.mult)

        ev = sm.tile([P, NT], F32, tag="ev", name="ev")
        v.tensor_scalar(ev[:, :], scores[:, :], 30.0, None, ALU.is_lt)
        dpi = sm.tile([P, NT], F32, tag="dpi", name="dpi")
        v.tensor_scalar(dpi[:, :], scores[:, :], 15.0, -15.0, ALU.max, ALU.add)

        u1 = sm.tile([P, NT], F32, tag="u1", name="u1")
        v.tensor_tensor(u1[:, :], ev[:, :], w1v, ALU.mult)
        v.tensor_tensor(u1[:, :], u1[:, :], e0v, ALU.add)
        ctot = sm.tile([P, NT], F32, tag="ctot", name="ctot")
        v.scalar_tensor_tensor(ctot[:, :], dpi[:, :], 2.0 / 30.0, u1[:, :],
                               ALU.mult, ALU.add)

        red = sm.tile([P, 1], F32, tag="red", name="red")
        v.tensor_reduce(red[:, :], ctot[:, :], axis=mybir.AxisListType.X,
                        op=ALU.add)

        # partition reduction on the (idle) tensor engine -> [1,1] PSUM
        ps = pp.tile([1, 1], F32, tag="ps", name="ps")
        nc.tensor.matmul(ps[:, :], red[:, :], constv(1.0), start=True, stop=True)
        outsb = sm.tile([1, 1], F32, tag="outsb", name="outsb")
        v.tensor_copy(outsb[:, :], ps[:, :])
        out_dma_holder.append(nc.sync.dma_start(out_h[:, :], outsb[:, :]))

    nc.finalize()
    return nc


def _get_nc() -> bass.Bass:
    global _NC_CACHE
    if _NC_CACHE is None:
        _NC_CACHE = _build_nc()
    return _NC_CACHE


def kernel(raw_sizes, raw_delays, raw_directions, delay_ms, padding_norm,
           confidence, profile_ids, trace=False, tmpdir=None):
    global LAST_RESULTS
    sz16 = np.asarray(raw_sizes).astype(np.float16)
    dl16 = np.asarray(raw_delays, dtype=np.float32).astype(np.float16)
    dr8 = np.asarray(raw_directions).astype(np.uint8)
    dms = np.asarray(delay_ms, dtype=np.float32)
    pad = np.asarray(padding_norm, dtype=np.float32)
    conf = np.asarray(confidence, dtype=np.float32)
    pid = np.asarray(profile_ids).astype(np.int64)

    td = TARGET_DELAY[pid]
    tp = TARGET_PAD[pid]
    mult = CONFIG_MULT[pid % 4]
    sim = np.abs(dms - td) + np.abs(pad - tp)
    eff = np.maximum(dms - 20.0, 0.0) / 20.0 + np.maximum(pad - 0.3, 0.0)
    vpack = np.zeros((B, 8), dtype=np.float32)
    vpack[:, 0] = pad * 1500.0
    vpack[:, 1] = dms
    vpack[:, 2] = mult * (100.0 / S)
    vpack[:, 3] = 0.5 * sim + 0.3 * eff + 0.2 * conf * conf
    vpack[:, 4] = 0.2 * (1.0 - 2.0 * conf)

    nc = _get_nc()
    in_maps = []
    for i in range(N_CORES):
        r = slice(i * BC, (i + 1) * BC)
        in_maps.append({
            "raw_sizes": sz16[r],
            "raw_delays": dl16[r],
            "raw_directions": dr8[r],
            "vpack": vpack[r],
        })

    LAST_RESULTS = run_bass_kernel_spmd(nc, in_maps, list(range(N_CORES)),
                                        trace=trace, tmpdir=tmpdir)
    partials = [LAST_RESULTS.results[i]["partial"] for i in range(N_CORES)]
    total = float(np.sum(np.stack(partials), dtype=np.float64))
    return np.float32(total / B)
